# revision 6
# baseline (speedup 1.0000x reference)
"""Trainium2 Bass kernel for nn_Bottleneck_75325136437765 (sparse 3x3 local attention bottleneck).

Sharding: data-parallel over batch B=16 across 8 cores (2 batches/core), params replicated.

Per-core layout: channels on partitions, spatial (32*32=1024) on free dim. All matmuls bf16,
fp32 PSUM accumulation.

v2: software-pipelined over the 2 per-core batches so PE never idles past the HAM
re-throttle window; e normalized at head level (epk * recip broadcast) so the v-phase
consumes one big per-mc broadcast instead of 18 small ones; output stored bf16.

  conv1/qkv/conv3: plain matmuls (lhsT = transposed weights, host-precomputed, bn scales folded).
  attention logits, packed PSUM layout (row = 32*(kk%4) + head, 3 tiles of 4/4/1 shifts):
      L[g,kk,hw] = sum_d q[gd,hw]*k[gd,hw+off_kk]  (col-tiled 0/1-selection matmuls over products)
                 + sum_d q[gd,hw]*pos[gd,kk]       (P2 matmul, accumulated into same PSUM)
  softmax over kk without max-subtraction:
      e = exp(L) (packed, 3 ACT ops); den = sum_kk e via 0/1 matmuls; recip = 1/den
      e_hat = e * recip (packed, recip cast+broadcast to packed rows via SWDGE)
      e_hat rearranged to head-level [32, 9, HW], then ONE broadcast DMA per channel chunk
      out_pre[c,hw] = sum_kk e_hat_bc[c,kk,hw] * v[c,hw+off_kk]
        per-shift product on DVE; sum over kk via identity-matmul PSUM accumulation
      h2 = relu(out_pre + bnatt_b)   (ACT, straight from PSUM)
  residual: identity matmul on bf16 x accumulated into the conv3 PSUM group.
"""

import numpy as np

import concourse.bass as bass
import concourse.bacc as bacc
import concourse.tile as tile
from concourse import mybir
from concourse.bass_utils import run_bass_kernel_spmd

# ---- problem constants (hardcoded per contract) ----
B, CIN, H, W = 16, 1024, 32, 32
WIDTH, OUT, HEADS, KS = 256, 1024, 32, 3
D = WIDTH // HEADS            # 8 channels per head
HW = H * W                    # 1024
NC_ = 8                       # cores
BL = B // NC_                 # 2 batches per core
P = 128
KC1 = CIN // P                # 8 contraction chunks for conv1
PT = WIDTH // P               # 2 partition tiles for width-256 tensors
OC = OUT // P                 # 8 output ptiles for conv3
NKK = KS * KS                 # 9 shifts
NT = 3                        # packed logit tiles (4+4+1 shifts)
F32 = mybir.dt.float32
BF16 = mybir.dt.bfloat16
NHALF = 2                     # PSUM-bank limit: matmul N<=512 fp32 out


def _ns(n):
    return slice(n * 512, (n + 1) * 512)


def build_program():
    nc = bacc.Bacc(None, target_bir_lowering=False, debug=False)

    def din(name, shape, dt=BF16):
        return nc.dram_tensor(name, list(shape), dt, kind="ExternalInput").ap()

    x16_d = din("x16", (BL, KC1, P, HW))
    w1T_d = din("w1T", (KC1, P, WIDTH))
    wqT_d = din("wqT", (PT, P, WIDTH))
    wkT_d = din("wkT", (PT, P, WIDTH))
    wvT_d = din("wvT", (PT, P, WIDTH))
    w3T_d = din("w3T", (PT, P, OUT))
    b1_d = din("b1", (PT, P, 1), F32)
    bq_d = din("bq", (PT, P, 1), F32)
    bk_d = din("bk", (PT, P, 1), F32)
    bv_d = din("bv", (PT, P, 1), F32)
    batt_d = din("batt", (PT, P, 1), F32)
    b3_d = din("b3", (OC, P, 1), F32)
    sel_d = din("sel", (PT, P, HEADS))
    p2_d = din("p2", (PT, P, NT, P))
    sab_d = din("sab", (P, HEADS))
    eye32_d = din("eye32", (HEADS, HEADS))
    ident_d = din("ident", (P, P))
    out_d = nc.dram_tensor("out", [BL, OC, P, HW], BF16, kind="ExternalOutput").ap()

    with tile.TileContext(nc) as tc:
        with (
            tc.tile_pool(name="consts", bufs=1) as consts,
            tc.tile_pool(name="xb", bufs=2) as xbp,
            tc.tile_pool(name="act", bufs=2) as actp,
            tc.tile_pool(name="pad", bufs=2) as padp,
            tc.tile_pool(name="epk", bufs=3) as epkp,
            tc.tile_pool(name="ehat", bufs=3) as ehatp,
            tc.tile_pool(name="ehead", bufs=2) as eheadp,
            tc.tile_pool(name="ebc", bufs=2) as ebcp,
            tc.tile_pool(name="rec", bufs=1) as recp,
            tc.tile_pool(name="tmp", bufs=6) as tmpp,
            tc.tile_pool(name="outz", bufs=4) as outzp,
            tc.tile_pool(name="pmm", bufs=2, space="PSUM") as pmm,
            tc.tile_pool(name="pL", bufs=1, space="PSUM") as pLp,
            tc.tile_pool(name="pacc", bufs=1, space="PSUM") as paccp,
        ):
            # ---- load constants ----
            # constants other than w1T/b1 go on the SWDGE queue so the sync
            # queue serves conv1's x/w chunks first (fast kernel start)
            def cload(name, dram, shape, dt=BF16, re="k p m -> p k m"):
                t = consts.tile(shape, dt, tag=name)
                nc.gpsimd.dma_start(out=t, in_=dram.rearrange(re) if re else dram)
                return t

            w1T = consts.tile([P, KC1, WIDTH], BF16, tag="w1T")
            b1 = consts.tile([P, PT, 1], F32, tag="b1")
            nc.sync.dma_start(out=b1, in_=b1_d.rearrange("k p m -> p k m"))
            wqT = cload("wqT", wqT_d, [P, PT, WIDTH])
            wkT = cload("wkT", wkT_d, [P, PT, WIDTH])
            wvT = cload("wvT", wvT_d, [P, PT, WIDTH])
            w3T = cload("w3T", w3T_d, [P, PT, OUT])
            bq = cload("bq", bq_d, [P, PT, 1], F32)
            bk = cload("bk", bk_d, [P, PT, 1], F32)
            bv = cload("bv", bv_d, [P, PT, 1], F32)
            batt = cload("batt", batt_d, [P, PT, 1], F32)
            b3 = cload("b3", b3_d, [P, OC, 1], F32)
            sel = cload("sel", sel_d, [P, PT, HEADS])
            p2 = cload("p2", p2_d, [P, PT, NT, P], re="k p m o -> p k m o")
            sab = cload("sab", sab_d, [P, HEADS], re=None)
            eye32 = cload("eye32", eye32_d, [HEADS, HEADS], re=None)
            ident = cload("ident", ident_d, [P, P], re=None)

            def head_bcast_ap(src16):
                # 2-level partition AP: dst[i*8+d] reads src partition (base+i)
                return bass.AP(tensor=src16.tensor, offset=src16.offset,
                               ap=[list(src16.ap[0]), [0, D]]
                                  + [list(a) for a in src16.ap[1:]])

            # persistent zero-padded k/v tiles, one pair per in-flight batch
            kpads, vpads = [], []
            for b in range(BL):
                kp = padp.tile([P, PT, H + 2, W + 2], BF16, tag="kpad")
                vp = padp.tile([P, PT, H + 2, W + 2], BF16, tag="vpad")
                nc.vector.memset(kp, 0.0)
                nc.vector.memset(vp, 0.0)
                kpads.append(kp)
                vpads.append(vp)

            # ---- per-batch state ----
            xbs = [None] * BL
            h1s = [None] * BL
            qs = [None] * BL
            epks = [[None] * NT for _ in range(BL)]
            h2s = [None] * BL
            eheads = [None] * BL
            ebcs = [[None] * PT for _ in range(BL)]

            def load_x(b):
                xb = xbp.tile([P, KC1, HW], BF16, tag="xb")
                xbs[b] = xb
                for kc in range(KC1):
                    if b == 0:
                        # separate HWDGE queue so w1T and x stream in parallel
                        nc.scalar.dma_start(out=w1T[:, kc, :], in_=w1T_d[kc])
                    nc.sync.dma_start(out=xb[:, kc, :], in_=x16_d[b, kc])

            def conv1(b):
                xb = xbs[b]
                h1 = actp.tile([P, PT, HW], BF16, tag="h1")
                h1s[b] = h1
                for mc in range(PT):
                    ps = pmm.tile([P, HW], F32, tag="mm")
                    for kc in range(KC1):
                        for n in range(NHALF):
                            nc.tensor.matmul(
                                ps[:, _ns(n)],
                                w1T[:, kc, mc * P:(mc + 1) * P],
                                xb[:, kc, _ns(n)],
                                start=(kc == 0), stop=(kc == KC1 - 1),
                            )
                    nc.scalar.activation(
                        out=h1[:, mc, :], in_=ps,
                        func=mybir.ActivationFunctionType.Relu,
                        bias=b1[:, mc], scale=1.0,
                    )

            def qkv(b):
                h1 = h1s[b]
                q = actp.tile([P, PT, HW], BF16, tag="q")
                qs[b] = q
                specs = [(wqT, bq, True, None), (wkT, bk, True, kpads[b]),
                         (wvT, bv, False, vpads[b])]
                for wT, bias, relu, dest in specs:
                    for mc in range(PT):
                        ps = pmm.tile([P, HW], F32, tag="mm")
                        for kc in range(PT):
                            for n in range(NHALF):
                                nc.tensor.matmul(
                                    ps[:, _ns(n)],
                                    wT[:, kc, mc * P:(mc + 1) * P],
                                    h1[:, kc, _ns(n)],
                                    start=(kc == 0), stop=(kc == PT - 1),
                                )
                        if dest is None:
                            o, i = q[:, mc, :], ps[:]
                        else:
                            o = dest[:, mc, 1:H + 1, 1:W + 1]
                            i = ps.rearrange("p (a b) -> p a b", a=H)
                        nc.scalar.activation(
                            out=o, in_=i,
                            func=(mybir.ActivationFunctionType.Relu if relu
                                  else mybir.ActivationFunctionType.Identity),
                            bias=bias[:, mc], scale=1.0,
                        )

            def logits_tile(b, t, denp):
                # packed tile t rows: 32*(kk%4) + g  for kk in {4t..4t+3}
                q, kpad = qs[b], kpads[b]
                nsh = 4 if t < 2 else 1
                rows = 32 * nsh
                Lpk = pLp.tile([P, HW], F32, tag="Lpk")
                # qpos term: all rows at once per pt chunk
                for n in range(NHALF):
                    for pt in range(PT):
                        nc.tensor.matmul(
                            Lpk[:rows, _ns(n)],
                            p2[:, pt, t, :rows],
                            q[:, pt, _ns(n)],
                            start=(pt == 0), stop=False,
                            skip_group_check=True,
                        )
                # qk products + col-tiled group reduce
                for j in range(nsh):
                    kk = 4 * t + j
                    di, dj = kk // KS, kk % KS
                    for pt in range(PT):
                        tmp = tmpp.tile([P, HW], BF16, tag="tmp")
                        nc.vector.tensor_tensor(
                            out=tmp.rearrange("p (a b) -> p a b", a=H),
                            in0=kpad[:, pt, di:di + H, dj:dj + W],
                            in1=q[:, pt, :].rearrange("p (a b) -> p a b", a=H),
                            op=mybir.AluOpType.mult,
                        )
                        for n in range(NHALF):
                            nc.tensor.matmul(
                                Lpk[32 * j:32 * (j + 1), _ns(n)],
                                sel[:, pt, :],
                                tmp[:, _ns(n)],
                                start=False, stop=(pt == PT - 1),
                                tile_position=(0, 32 * j),
                                skip_group_check=True,
                            )
                epk = epkp.tile([P, HW], BF16, tag="epk")
                nc.scalar.activation(
                    out=epk[:rows, :], in_=Lpk[:rows, :],
                    func=mybir.ActivationFunctionType.Exp,
                )
                epks[b][t] = epk
                # denominator accumulation
                lhs = sab if t < 2 else eye32
                for n in range(NHALF):
                    nc.tensor.matmul(
                        denp[:, _ns(n)], lhs[:rows, :], epk[:rows, _ns(n)],
                        start=(t == 0), stop=(t == NT - 1),
                        skip_group_check=True,
                    )

            def softchain(b):
                # recip -> cast+bcast to packed rows -> ehat -> head-level
                # rearrange -> one big per-mc channel broadcast
                denp = softchain.denps[b]
                rec32 = recp.tile([HEADS, HW], F32, tag="rec32")
                nc.vector.reciprocal_approx_fast(out=rec32, in_=denp)
                recpk = recp.tile([P, HW], BF16, tag="recpk")
                for j in range(4):
                    # SWDGE: fp32 -> bf16 cast during DMA
                    nc.gpsimd.dma_start(out=recpk[32 * j:32 * (j + 1), :],
                                        in_=rec32)
                ehead = eheadp.tile([HEADS, NKK, HW], BF16, tag="ehead")
                eheads[b] = ehead
                for t in range(NT):
                    nsh = 4 if t < 2 else 1
                    rows = 32 * nsh
                    ehat = ehatp.tile([P, HW], BF16, tag="ehat")
                    nc.vector.tensor_tensor(
                        out=ehat[:rows, :], in0=epks[b][t][:rows, :],
                        in1=recpk[:rows, :], op=mybir.AluOpType.mult,
                    )
                    for j in range(nsh):
                        eng = nc.sync if (t + j) % 2 == 0 else nc.scalar
                        eng.dma_start(out=ehead[:, 4 * t + j, :],
                                      in_=ehat[32 * j:32 * (j + 1), :])
                for mc in range(PT):
                    ebc = ebcp.tile([P, NKK, HW], BF16, tag="ebc")
                    ebcs[b][mc] = ebc
                    eng = nc.sync if mc == 0 else nc.scalar
                    eng.dma_start(
                        out=ebc,
                        in_=head_bcast_ap(ehead[16 * mc:16 * (mc + 1), :, :]))

            softchain.denps = [None] * BL

            def den_alloc(b):
                denp = pmm.tile([HEADS, HW], F32, tag="mm")
                softchain.denps[b] = denp
                return denp

            def ev_chunk(b, mc):
                # out_pre[c] = sum_kk ehat_bc * v_shift for one 128-chan chunk
                vpad = vpads[b]
                acc = paccp.tile([P, HW], F32, tag="acc")
                for kk in range(NKK):
                    di, dj = kk // KS, kk % KS
                    t2 = tmpp.tile([P, HW], BF16, tag="tmp")
                    nc.vector.tensor_tensor(
                        out=t2.rearrange("p (a b) -> p a b", a=H),
                        in0=ebcs[b][mc][:, kk, :].rearrange("p (a b) -> p a b", a=H),
                        in1=vpad[:, mc, di:di + H, dj:dj + W],
                        op=mybir.AluOpType.mult,
                    )
                    for n in range(NHALF):
                        nc.tensor.matmul(
                            acc[:, _ns(n)], ident, t2[:, _ns(n)],
                            start=(kk == 0), stop=(kk == NKK - 1),
                            skip_group_check=True,
                        )
                if mc == 0:
                    h2 = actp.tile([P, PT, HW], BF16, tag="h2")
                    h2s[b] = h2
                nc.scalar.activation(
                    out=h2s[b][:, mc, :], in_=acc,
                    func=mybir.ActivationFunctionType.Relu,
                    bias=batt[:, mc], scale=1.0,
                )

            def conv3(b, ocs):
                h2, xb = h2s[b], xbs[b]
                for oc in ocs:
                    ps = pmm.tile([P, HW], F32, tag="mm")
                    for n in range(NHALF):
                        for kc in range(PT):
                            nc.tensor.matmul(
                                ps[:, _ns(n)],
                                w3T[:, kc, oc * P:(oc + 1) * P],
                                h2[:, kc, _ns(n)],
                                start=(kc == 0), stop=False,
                                skip_group_check=True,
                            )
                        nc.tensor.matmul(
                            ps[:, _ns(n)], ident, xb[:, oc, _ns(n)],
                            start=False, stop=True,
                            skip_group_check=True,
                        )
                    zr = outzp.tile([P, HW], BF16, tag="outzr")
                    nc.scalar.activation(
                        out=zr, in_=ps, func=mybir.ActivationFunctionType.Relu,
                        bias=b3[:, oc], scale=1.0,
                    )
                    eng = nc.scalar if oc % 2 == 0 else nc.sync
                    eng.dma_start(out=out_d[b, oc], in_=zr)

            # ---- pipelined schedule over the two batches ----
            load_x(0)
            load_x(1)
            conv1(0)
            qkv(0)
            # b0 logits (pipeline fill: nothing to overlap yet)
            d0 = den_alloc(0)
            for t in range(NT):
                logits_tile(0, t, d0)
            # b0 softmax chain runs on DVE/DMA while PE does b1 convs
            softchain(0)
            conv1(1)
            qkv(1)
            # b0 v-phase (DVE+PE) interleaved with b1 logits (DVE+PE)
            d1 = den_alloc(1)
            ev_chunk(0, 0)
            logits_tile(1, 0, d1)
            ev_chunk(0, 1)
            logits_tile(1, 1, d1)
            logits_tile(1, 2, d1)
            # b1 softmax chain on DVE/DMA while PE does b0 conv3
            softchain(1)
            conv3(0, range(OC))
            # b1 v-phase, then b1 conv3 (tail)
            ev_chunk(1, 0)
            ev_chunk(1, 1)
            conv3(1, range(OC))

    nc.compile()
    return nc


_PROG = None


def _host_prep(inputs):
    import ml_dtypes
    bf = ml_dtypes.bfloat16
    f = lambda a: np.asarray(a, dtype=np.float32)
    x = f(inputs["x"])
    # fold bn scales into weights (bn(conv(x,W),s,b) = conv(x, s*W) + b)
    w1 = f(inputs["w_conv1"]) * f(inputs["bn1_s"])[:, None]
    wq = f(inputs["wq"]) * f(inputs["bnq_s"])[:, None]
    wk = f(inputs["wk"]) * f(inputs["bnk_s"])[:, None]
    # fold bnatt scale through the (linear) attention-value path into v
    sv = f(inputs["bnatt_s"]) * f(inputs["bnv_s"])
    wv = f(inputs["wv"]) * sv[:, None]
    bv = f(inputs["bnatt_s"]) * f(inputs["bnv_b"])
    w3 = f(inputs["w_conv3"]) * f(inputs["bn3_s"])[:, None]

    posf = (f(inputs["pos_h"]) + f(inputs["pos_w"])).reshape(WIDTH, NKK)

    sel = np.zeros((PT, P, HEADS), np.float32)
    for pt in range(PT):
        for c in range(P):
            sel[pt, c, pt * (P // D) + c // D] = 1.0
    # p2[pt, c, t, 32*j+g] = pos[c_global, 4t+j] if head(c_global)==g
    p2 = np.zeros((PT, P, NT, P), np.float32)
    for pt in range(PT):
        for c in range(P):
            g = pt * (P // D) + c // D
            for kk in range(NKK):
                t, j = kk // 4, kk % 4
                p2[pt, c, t, 32 * j + g] = posf[pt * P + c, kk]
    # sab[r, g] = 1 if r % 32 == g (sum over the 4 packed kk rows)
    sab = np.zeros((P, HEADS), np.float32)
    for r in range(P):
        sab[r, r % HEADS] = 1.0
    com = {
        "w1T": np.ascontiguousarray(w1.T.reshape(KC1, P, WIDTH)).astype(bf),
        "wqT": np.ascontiguousarray(wq.T.reshape(PT, P, WIDTH)).astype(bf),
        "wkT": np.ascontiguousarray(wk.T.reshape(PT, P, WIDTH)).astype(bf),
        "wvT": np.ascontiguousarray(wv.T.reshape(PT, P, WIDTH)).astype(bf),
        "w3T": np.ascontiguousarray(w3.T.reshape(PT, P, OUT)).astype(bf),
        "b1": f(inputs["bn1_b"]).reshape(PT, P, 1),
        "bq": f(inputs["bnq_b"]).reshape(PT, P, 1),
        "bk": f(inputs["bnk_b"]).reshape(PT, P, 1),
        "bv": bv.reshape(PT, P, 1),
        "batt": f(inputs["bnatt_b"]).reshape(PT, P, 1),
        "b3": f(inputs["bn3_b"]).reshape(OC, P, 1),
        "sel": sel.astype(bf),
        "p2": p2.astype(bf),
        "sab": sab.astype(bf),
        "eye32": np.eye(HEADS, dtype=np.float32).astype(bf),
        "ident": np.eye(P, dtype=np.float32).astype(bf),
    }
    xr = x.reshape(B, KC1, P, HW)
    in_maps = []
    for c in range(NC_):
        xs = np.ascontiguousarray(xr[c * BL:(c + 1) * BL])
        in_maps.append(dict(com, x16=xs.astype(bf)))
    return in_maps


def kernel(**inputs):
    global _PROG
    if _PROG is None:
        _PROG = build_program()
    in_maps = _host_prep(inputs)
    res = run_bass_kernel_spmd(_PROG, in_maps, core_ids=list(range(NC_)))
    outs = [np.asarray(res.results[c]["out"], dtype=np.float32)
            .reshape(BL, OUT, H, W) for c in range(NC_)]
    return np.concatenate(outs, axis=0)


# revision 10
# speedup vs baseline: 1.2477x; 1.2477x over previous
"""Trainium2 Bass kernel for nn_Bottleneck_75325136437765 (sparse 3x3 local attention bottleneck).

Sharding: data-parallel over batch B=16 across 8 cores (2 batches/core), params replicated.

Per-core layout: channels on partitions, spatial (32*32=1024) on free dim. All matmuls bf16,
fp32 PSUM accumulation.

v2: software-pipelined over the 2 per-core batches so PE never idles past the HAM
re-throttle window; e normalized at head level (epk * recip broadcast) so the v-phase
consumes one big per-mc broadcast instead of 18 small ones; output stored bf16.

  conv1/qkv/conv3: plain matmuls (lhsT = transposed weights, host-precomputed, bn scales folded).
  attention logits, packed PSUM layout (row = 32*(kk%4) + head, 3 tiles of 4/4/1 shifts):
      L[g,kk,hw] = sum_d q[gd,hw]*k[gd,hw+off_kk]  (col-tiled 0/1-selection matmuls over products)
                 + sum_d q[gd,hw]*pos[gd,kk]       (P2 matmul, accumulated into same PSUM)
  softmax over kk without max-subtraction:
      e = exp(L) (packed, 3 ACT ops); den = sum_kk e via 0/1 matmuls; recip = 1/den
      e_hat = e * recip (packed, recip cast+broadcast to packed rows via SWDGE)
      e_hat rearranged to head-level [32, 9, HW], then ONE broadcast DMA per channel chunk
      out_pre[c,hw] = sum_kk e_hat_bc[c,kk,hw] * v[c,hw+off_kk]
        per-shift product on DVE; sum over kk via identity-matmul PSUM accumulation
      h2 = relu(out_pre + bnatt_b)   (ACT, straight from PSUM)
  residual: identity matmul on bf16 x accumulated into the conv3 PSUM group.
"""

import numpy as np

import concourse.bass as bass
import concourse.bacc as bacc
import concourse.tile as tile
from concourse import mybir
from concourse.bass_utils import run_bass_kernel_spmd

# ---- problem constants (hardcoded per contract) ----
B, CIN, H, W = 16, 1024, 32, 32
WIDTH, OUT, HEADS, KS = 256, 1024, 32, 3
D = WIDTH // HEADS            # 8 channels per head
HW = H * W                    # 1024
NC_ = 8                       # cores
BL = B // NC_                 # 2 batches per core
P = 128
KC1 = CIN // P                # 8 contraction chunks for conv1
PT = WIDTH // P               # 2 partition tiles for width-256 tensors
OC = OUT // P                 # 8 output ptiles for conv3
NKK = KS * KS                 # 9 shifts
NT = 3                        # packed logit tiles (4+4+1 shifts)
F32 = mybir.dt.float32
BF16 = mybir.dt.bfloat16
NHALF = 2                     # PSUM-bank limit: matmul N<=512 fp32 out


def _ns(n):
    return slice(n * 512, (n + 1) * 512)


def build_program():
    nc = bacc.Bacc(None, target_bir_lowering=False, debug=False)

    def din(name, shape, dt=BF16):
        return nc.dram_tensor(name, list(shape), dt, kind="ExternalInput").ap()

    x16_d = din("x16", (BL, KC1, P, HW))
    w1T_d = din("w1T", (KC1, P, WIDTH))
    wqT_d = din("wqT", (PT, P, WIDTH))
    wkT_d = din("wkT", (PT, P, WIDTH))
    wvT_d = din("wvT", (PT, P, WIDTH))
    w3T_d = din("w3T", (PT, P, OUT))
    b1_d = din("b1", (PT, P, 1), F32)
    bq_d = din("bq", (PT, P, 1), F32)
    bk_d = din("bk", (PT, P, 1), F32)
    bv_d = din("bv", (PT, P, 1), F32)
    batt_d = din("batt", (PT, P, 1), F32)
    b3_d = din("b3", (OC, P, 1), F32)
    sel_d = din("sel", (PT, P, HEADS))
    p2_d = din("p2", (PT, P, NT, P))
    sab_d = din("sab", (P, HEADS))
    eye32_d = din("eye32", (HEADS, HEADS))
    ident_d = din("ident", (P, P))
    bm_d = din("bm", (PT, 4, P, P))
    out_d = nc.dram_tensor("out", [BL, OC, P, HW], BF16, kind="ExternalOutput").ap()

    with tile.TileContext(nc) as tc:
        with (
            tc.tile_pool(name="consts", bufs=1) as consts,
            tc.tile_pool(name="xb", bufs=2) as xbp,
            tc.tile_pool(name="act", bufs=2) as actp,
            tc.tile_pool(name="pad", bufs=2) as padp,
            tc.tile_pool(name="epk", bufs=3) as epkp,
            tc.tile_pool(name="ehat", bufs=3) as ehatp,
            tc.tile_pool(name="rec", bufs=1) as recp,
            tc.tile_pool(name="tmp", bufs=8) as tmpp,
            tc.tile_pool(name="tmph", bufs=8) as tmphp,
            tc.tile_pool(name="outz", bufs=4) as outzp,
            tc.tile_pool(name="pmm", bufs=2, space="PSUM") as pmm,
            tc.tile_pool(name="pacc", bufs=1, space="PSUM") as paccp,
            tc.tile_pool(name="pebc", bufs=2, space="PSUM") as pebcp,
        ):
            # ---- load constants ----
            # constants other than w1T/b1 go on the SWDGE queue so the sync
            # queue serves conv1's x/w chunks first (fast kernel start)
            def cload(name, dram, shape, dt=BF16, re="k p m -> p k m"):
                t = consts.tile(shape, dt, tag=name)
                nc.gpsimd.dma_start(out=t, in_=dram.rearrange(re) if re else dram)
                return t

            w1T = consts.tile([P, KC1, WIDTH], BF16, tag="w1T")
            b1 = consts.tile([P, PT, 1], F32, tag="b1")
            nc.sync.dma_start(out=b1, in_=b1_d.rearrange("k p m -> p k m"))
            wqT = cload("wqT", wqT_d, [P, PT, WIDTH])
            wkT = cload("wkT", wkT_d, [P, PT, WIDTH])
            wvT = cload("wvT", wvT_d, [P, PT, WIDTH])
            w3T = cload("w3T", w3T_d, [P, PT, OUT])
            bq = cload("bq", bq_d, [P, PT, 1], F32)
            bk = cload("bk", bk_d, [P, PT, 1], F32)
            bv = cload("bv", bv_d, [P, PT, 1], F32)
            batt = cload("batt", batt_d, [P, PT, 1], F32)
            b3 = cload("b3", b3_d, [P, OC, 1], F32)
            sel = cload("sel", sel_d, [P, PT, HEADS])
            p2 = cload("p2", p2_d, [P, PT, NT, P], re="k p m o -> p k m o")
            sab = cload("sab", sab_d, [P, HEADS], re=None)
            eye32 = cload("eye32", eye32_d, [HEADS, HEADS], re=None)
            ident = cload("ident", ident_d, [P, P], re=None)
            bm = cload("bm", bm_d, [P, PT, 4, P], re="m j r c -> r m j c")

            def head_bcast_ap(src16):
                # 2-level partition AP: dst[i*8+d] reads src partition (base+i)
                return bass.AP(tensor=src16.tensor, offset=src16.offset,
                               ap=[list(src16.ap[0]), [0, D]]
                                  + [list(a) for a in src16.ap[1:]])

            # persistent zero-padded k/v tiles, one pair per in-flight batch
            kpads, vpads = [], []
            for b in range(BL):
                kp = padp.tile([P, PT, H + 2, W + 2], BF16, tag="kpad")
                vp = padp.tile([P, PT, H + 2, W + 2], BF16, tag="vpad")
                nc.vector.memset(kp, 0.0)
                nc.vector.memset(vp, 0.0)
                kpads.append(kp)
                vpads.append(vp)

            # ---- per-batch state ----
            xbs = [None] * BL
            h1s = [None] * BL
            qs = [None] * BL
            epks = [[None] * NT for _ in range(BL)]
            h2s = [None] * BL

            def load_x(b):
                xb = xbp.tile([P, KC1, HW], BF16, tag="xb")
                xbs[b] = xb
                for kc in range(KC1):
                    if b == 0:
                        # separate HWDGE queue so w1T and x stream in parallel
                        nc.scalar.dma_start(out=w1T[:, kc, :], in_=w1T_d[kc])
                    nc.sync.dma_start(out=xb[:, kc, :], in_=x16_d[b, kc])

            def conv1(b):
                xb = xbs[b]
                h1 = actp.tile([P, PT, HW], BF16, tag="h1")
                h1s[b] = h1
                for mc in range(PT):
                    ps = pmm.tile([P, HW], F32, tag="mm")
                    for kc in range(KC1):
                        for n in range(NHALF):
                            nc.tensor.matmul(
                                ps[:, _ns(n)],
                                w1T[:, kc, mc * P:(mc + 1) * P],
                                xb[:, kc, _ns(n)],
                                start=(kc == 0), stop=(kc == KC1 - 1),
                            )
                    nc.scalar.activation(
                        out=h1[:, mc, :], in_=ps,
                        func=mybir.ActivationFunctionType.Relu,
                        bias=b1[:, mc], scale=1.0,
                    )

            def qkv(b):
                h1 = h1s[b]
                q = actp.tile([P, PT, HW], BF16, tag="q")
                qs[b] = q
                specs = [(wqT, bq, True, None), (wkT, bk, True, kpads[b]),
                         (wvT, bv, False, vpads[b])]
                for wT, bias, relu, dest in specs:
                    for mc in range(PT):
                        ps = pmm.tile([P, HW], F32, tag="mm")
                        for kc in range(PT):
                            for n in range(NHALF):
                                nc.tensor.matmul(
                                    ps[:, _ns(n)],
                                    wT[:, kc, mc * P:(mc + 1) * P],
                                    h1[:, kc, _ns(n)],
                                    start=(kc == 0), stop=(kc == PT - 1),
                                )
                        if dest is None:
                            o, i = q[:, mc, :], ps[:]
                        else:
                            o = dest[:, mc, 1:H + 1, 1:W + 1]
                            i = ps.rearrange("p (a b) -> p a b", a=H)
                        nc.scalar.activation(
                            out=o, in_=i,
                            func=(mybir.ActivationFunctionType.Relu if relu
                                  else mybir.ActivationFunctionType.Identity),
                            bias=bias[:, mc], scale=1.0,
                        )

            def logits_tile(b, t):
                # packed tile t rows: 32*(kk%4) + g  for kk in {4t..4t+3}
                q, kpad = qs[b], kpads[b]
                nsh = 4 if t < 2 else 1
                rows = 32 * nsh
                Lpk = pmm.tile([P, HW], F32, tag="mm")
                # qpos term: all rows at once per pt chunk
                for n in range(NHALF):
                    for pt in range(PT):
                        nc.tensor.matmul(
                            Lpk[:rows, _ns(n)],
                            p2[:, pt, t, :rows],
                            q[:, pt, _ns(n)],
                            start=(pt == 0), stop=False,
                            skip_group_check=True,
                        )
                # qk products + col-tiled group reduce
                for j in range(nsh):
                    kk = 4 * t + j
                    di, dj = kk // KS, kk % KS
                    for pt in range(PT):
                        tmp = tmpp.tile([P, HW], BF16, tag="tmp")
                        nc.vector.tensor_tensor(
                            out=tmp.rearrange("p (a b) -> p a b", a=H),
                            in0=kpad[:, pt, di:di + H, dj:dj + W],
                            in1=q[:, pt, :].rearrange("p (a b) -> p a b", a=H),
                            op=mybir.AluOpType.mult,
                        )
                        for n in range(NHALF):
                            nc.tensor.matmul(
                                Lpk[32 * j:32 * (j + 1), _ns(n)],
                                sel[:, pt, :],
                                tmp[:, _ns(n)],
                                start=False, stop=(pt == PT - 1),
                                tile_position=(0, 32 * j),
                                skip_group_check=True,
                            )
                epk = epkp.tile([P, HW], BF16, tag="epk")
                nc.scalar.activation(
                    out=epk[:rows, :], in_=Lpk[:rows, :],
                    func=mybir.ActivationFunctionType.Exp,
                )
                epks[b][t] = epk

            ehats = [[None] * NT for _ in range(BL)]

            def softchain(b):
                # denominator (3 back-to-back PSUM-accumulated matmuls per
                # half), recip, recip cast+bcast to packed rows, ehat mults
                denp = pmm.tile([HEADS, HW], F32, tag="mm")
                for n in range(NHALF):
                    for t in range(NT):
                        rows = 128 if t < 2 else 32
                        lhs = sab if t < 2 else eye32
                        nc.tensor.matmul(
                            denp[:, _ns(n)], lhs[:rows, :],
                            epks[b][t][:rows, _ns(n)],
                            start=(t == 0), stop=(t == NT - 1),
                            skip_group_check=True,
                        )
                rec32 = recp.tile([HEADS, HW], F32, tag="rec32")
                nc.vector.reciprocal_approx_fast(out=rec32, in_=denp)
                recpk = recp.tile([P, HW], BF16, tag="recpk")
                for j in range(4):
                    # SWDGE: fp32 -> bf16 cast during DMA
                    nc.gpsimd.dma_start(out=recpk[32 * j:32 * (j + 1), :],
                                        in_=rec32)
                for t in range(NT):
                    rows = 128 if t < 2 else 32
                    ehat = ehatp.tile([P, HW], BF16, tag="ehat")
                    nc.vector.tensor_tensor(
                        out=ehat[:rows, :], in0=epks[b][t][:rows, :],
                        in1=recpk[:rows, :], op=mybir.AluOpType.mult,
                    )
                    ehats[b][t] = ehat

            def ev_chunk(b, mc):
                # out_pre[c] = sum_kk ehat_bc * v_shift for one 128-chan chunk
                # ehat broadcast head->channels via 0/1 matmul into PSUM halves
                vpad = vpads[b]
                acc = paccp.tile([P, HW], F32, tag="acc")
                HH = H // NHALF
                for kk in range(NKK):
                    t, j = kk // 4, kk % 4
                    di, dj = kk // KS, kk % KS
                    rows = 128 if t < 2 else 32
                    for n in range(NHALF):
                        eb = pebcp.tile([P, 512], F32, tag="eb")
                        nc.tensor.matmul(
                            eb, bm[:rows, mc, j, :], ehats[b][t][:rows, _ns(n)],
                            start=True, stop=True,
                            skip_group_check=True,
                        )
                        t2 = tmphp.tile([P, 512], BF16, tag="tmph")
                        a0 = di + HH * n
                        nc.vector.tensor_tensor(
                            out=t2.rearrange("p (a b) -> p a b", a=HH),
                            in0=eb.rearrange("p (a b) -> p a b", a=HH),
                            in1=vpad[:, mc, a0:a0 + HH, dj:dj + W],
                            op=mybir.AluOpType.mult,
                        )
                        nc.tensor.matmul(
                            acc[:, _ns(n)], ident, t2,
                            start=(kk == 0), stop=(kk == NKK - 1),
                            skip_group_check=True,
                        )
                if mc == 0:
                    h2 = actp.tile([P, PT, HW], BF16, tag="h2")
                    h2s[b] = h2
                nc.scalar.activation(
                    out=h2s[b][:, mc, :], in_=acc,
                    func=mybir.ActivationFunctionType.Relu,
                    bias=batt[:, mc], scale=1.0,
                )

            def conv3(b, ocs):
                h2, xb = h2s[b], xbs[b]
                for oc in ocs:
                    ps = pmm.tile([P, HW], F32, tag="mm")
                    for n in range(NHALF):
                        for kc in range(PT):
                            nc.tensor.matmul(
                                ps[:, _ns(n)],
                                w3T[:, kc, oc * P:(oc + 1) * P],
                                h2[:, kc, _ns(n)],
                                start=(kc == 0), stop=False,
                                skip_group_check=True,
                            )
                        nc.tensor.matmul(
                            ps[:, _ns(n)], ident, xb[:, oc, _ns(n)],
                            start=False, stop=True,
                            skip_group_check=True,
                        )
                    zr = outzp.tile([P, HW], BF16, tag="outzr")
                    nc.scalar.activation(
                        out=zr, in_=ps, func=mybir.ActivationFunctionType.Relu,
                        bias=b3[:, oc], scale=1.0,
                    )
                    eng = nc.scalar if oc % 2 == 0 else nc.sync
                    eng.dma_start(out=out_d[b, oc], in_=zr)

            # ---- pipelined schedule over the two batches ----
            load_x(0)
            load_x(1)
            conv1(0)
            qkv(0)
            # b0 logits (pipeline fill: nothing to overlap yet)
            for t in range(NT):
                logits_tile(0, t)
            # b0 softmax chain runs on DVE/DMA while PE does b1 convs
            softchain(0)
            conv1(1)
            qkv(1)
            # b0 v-phase (DVE+PE) interleaved with b1 logits (DVE+PE)
            ev_chunk(0, 0)
            logits_tile(1, 0)
            ev_chunk(0, 1)
            logits_tile(1, 1)
            logits_tile(1, 2)
            # b1 softmax chain on DVE/DMA while PE does b0 conv3
            softchain(1)
            conv3(0, range(OC))
            # b1 v-phase, then b1 conv3 (tail)
            ev_chunk(1, 0)
            ev_chunk(1, 1)
            conv3(1, range(OC))

    nc.compile()
    return nc


_PROG = None


def _host_prep(inputs):
    import ml_dtypes
    bf = ml_dtypes.bfloat16
    f = lambda a: np.asarray(a, dtype=np.float32)
    x = f(inputs["x"])
    # fold bn scales into weights (bn(conv(x,W),s,b) = conv(x, s*W) + b)
    w1 = f(inputs["w_conv1"]) * f(inputs["bn1_s"])[:, None]
    wq = f(inputs["wq"]) * f(inputs["bnq_s"])[:, None]
    wk = f(inputs["wk"]) * f(inputs["bnk_s"])[:, None]
    # fold bnatt scale through the (linear) attention-value path into v
    sv = f(inputs["bnatt_s"]) * f(inputs["bnv_s"])
    wv = f(inputs["wv"]) * sv[:, None]
    bv = f(inputs["bnatt_s"]) * f(inputs["bnv_b"])
    w3 = f(inputs["w_conv3"]) * f(inputs["bn3_s"])[:, None]

    posf = (f(inputs["pos_h"]) + f(inputs["pos_w"])).reshape(WIDTH, NKK)

    sel = np.zeros((PT, P, HEADS), np.float32)
    for pt in range(PT):
        for c in range(P):
            sel[pt, c, pt * (P // D) + c // D] = 1.0
    # p2[pt, c, t, 32*j+g] = pos[c_global, 4t+j] if head(c_global)==g
    p2 = np.zeros((PT, P, NT, P), np.float32)
    for pt in range(PT):
        for c in range(P):
            g = pt * (P // D) + c // D
            for kk in range(NKK):
                t, j = kk // 4, kk % 4
                p2[pt, c, t, 32 * j + g] = posf[pt * P + c, kk]
    # sab[r, g] = 1 if r % 32 == g (sum over the 4 packed kk rows)
    sab = np.zeros((P, HEADS), np.float32)
    for r in range(P):
        sab[r, r % HEADS] = 1.0
    # bm[mc, j, r, c] = 1 if r == 32*j + 16*mc + c//8 (head->channel bcast)
    bm = np.zeros((PT, 4, P, P), np.float32)
    for mc in range(PT):
        for j in range(4):
            for c in range(P):
                bm[mc, j, 32 * j + 16 * mc + c // D, c] = 1.0
    com = {
        "w1T": np.ascontiguousarray(w1.T.reshape(KC1, P, WIDTH)).astype(bf),
        "wqT": np.ascontiguousarray(wq.T.reshape(PT, P, WIDTH)).astype(bf),
        "wkT": np.ascontiguousarray(wk.T.reshape(PT, P, WIDTH)).astype(bf),
        "wvT": np.ascontiguousarray(wv.T.reshape(PT, P, WIDTH)).astype(bf),
        "w3T": np.ascontiguousarray(w3.T.reshape(PT, P, OUT)).astype(bf),
        "b1": f(inputs["bn1_b"]).reshape(PT, P, 1),
        "bq": f(inputs["bnq_b"]).reshape(PT, P, 1),
        "bk": f(inputs["bnk_b"]).reshape(PT, P, 1),
        "bv": bv.reshape(PT, P, 1),
        "batt": f(inputs["bnatt_b"]).reshape(PT, P, 1),
        "b3": f(inputs["bn3_b"]).reshape(OC, P, 1),
        "sel": sel.astype(bf),
        "p2": p2.astype(bf),
        "sab": sab.astype(bf),
        "eye32": np.eye(HEADS, dtype=np.float32).astype(bf),
        "ident": np.eye(P, dtype=np.float32).astype(bf),
        "bm": bm.astype(bf),
    }
    xr = x.reshape(B, KC1, P, HW)
    in_maps = []
    for c in range(NC_):
        xs = np.ascontiguousarray(xr[c * BL:(c + 1) * BL])
        in_maps.append(dict(com, x16=xs.astype(bf)))
    return in_maps


def kernel(**inputs):
    global _PROG
    if _PROG is None:
        _PROG = build_program()
    in_maps = _host_prep(inputs)
    res = run_bass_kernel_spmd(_PROG, in_maps, core_ids=list(range(NC_)))
    outs = [np.asarray(res.results[c]["out"], dtype=np.float32)
            .reshape(BL, OUT, H, W) for c in range(NC_)]
    return np.concatenate(outs, axis=0)


# revision 12
# speedup vs baseline: 1.2768x; 1.0233x over previous
"""Trainium2 Bass kernel for nn_Bottleneck_75325136437765 (sparse 3x3 local attention bottleneck).

Sharding: data-parallel over batch B=16 across 8 cores (2 batches/core), params replicated.

Per-core layout: channels on partitions, spatial (32*32=1024) on free dim. All matmuls bf16,
fp32 PSUM accumulation.

v2: software-pipelined over the 2 per-core batches so PE never idles past the HAM
re-throttle window; e normalized at head level (epk * recip broadcast) so the v-phase
consumes one big per-mc broadcast instead of 18 small ones; output stored bf16.

  conv1/qkv/conv3: plain matmuls (lhsT = transposed weights, host-precomputed, bn scales folded).
  attention logits, packed PSUM layout (row = 32*(kk%4) + head, 3 tiles of 4/4/1 shifts):
      L[g,kk,hw] = sum_d q[gd,hw]*k[gd,hw+off_kk]  (col-tiled 0/1-selection matmuls over products)
                 + sum_d q[gd,hw]*pos[gd,kk]       (P2 matmul, accumulated into same PSUM)
  softmax over kk without max-subtraction:
      e = exp(L) (packed, 3 ACT ops); den = sum_kk e via 0/1 matmuls; recip = 1/den
      e_hat = e * recip (packed, recip cast+broadcast to packed rows via SWDGE)
      e_hat rearranged to head-level [32, 9, HW], then ONE broadcast DMA per channel chunk
      out_pre[c,hw] = sum_kk e_hat_bc[c,kk,hw] * v[c,hw+off_kk]
        per-shift product on DVE; sum over kk via identity-matmul PSUM accumulation
      h2 = relu(out_pre + bnatt_b)   (ACT, straight from PSUM)
  residual: identity matmul on bf16 x accumulated into the conv3 PSUM group.
"""

import numpy as np

import concourse.bass as bass
import concourse.bacc as bacc
import concourse.tile as tile
from concourse import mybir
from concourse.bass_utils import run_bass_kernel_spmd

# ---- problem constants (hardcoded per contract) ----
B, CIN, H, W = 16, 1024, 32, 32
WIDTH, OUT, HEADS, KS = 256, 1024, 32, 3
D = WIDTH // HEADS            # 8 channels per head
HW = H * W                    # 1024
NC_ = 8                       # cores
BL = B // NC_                 # 2 batches per core
P = 128
KC1 = CIN // P                # 8 contraction chunks for conv1
PT = WIDTH // P               # 2 partition tiles for width-256 tensors
OC = OUT // P                 # 8 output ptiles for conv3
NKK = KS * KS                 # 9 shifts
NT = 3                        # packed logit tiles (4+4+1 shifts)
F32 = mybir.dt.float32
BF16 = mybir.dt.bfloat16
NHALF = 2                     # PSUM-bank limit: matmul N<=512 fp32 out


def _ns(n):
    return slice(n * 512, (n + 1) * 512)


def build_program():
    nc = bacc.Bacc(None, target_bir_lowering=False, debug=False)

    def din(name, shape, dt=BF16):
        return nc.dram_tensor(name, list(shape), dt, kind="ExternalInput").ap()

    x16_d = din("x16", (BL, KC1, P, HW))
    w1T_d = din("w1T", (KC1, P, WIDTH))
    wqT_d = din("wqT", (PT, P, WIDTH))
    wkT_d = din("wkT", (PT, P, WIDTH))
    wvT_d = din("wvT", (PT, P, WIDTH))
    w3T_d = din("w3T", (PT, P, OUT))
    b1_d = din("b1", (PT, P, 1), F32)
    bq_d = din("bq", (PT, P, 1), F32)
    bk_d = din("bk", (PT, P, 1), F32)
    bv_d = din("bv", (PT, P, 1), F32)
    batt_d = din("batt", (PT, P, 1), F32)
    b3_d = din("b3", (OC, P, 1), F32)
    sel_d = din("sel", (PT, P, HEADS))
    p2_d = din("p2", (PT, P, NT, P))
    sab_d = din("sab", (P, HEADS))
    eye32_d = din("eye32", (HEADS, HEADS))
    ident_d = din("ident", (P, P))
    bm_d = din("bm", (PT, 4, P, P))
    out_d = nc.dram_tensor("out", [BL, OC, P, HW], BF16, kind="ExternalOutput").ap()

    with tile.TileContext(nc) as tc:
        with (
            tc.tile_pool(name="consts", bufs=1) as consts,
            tc.tile_pool(name="xb", bufs=2) as xbp,
            tc.tile_pool(name="act", bufs=2) as actp,
            tc.tile_pool(name="pad", bufs=2) as padp,
            tc.tile_pool(name="epk", bufs=3) as epkp,
            tc.tile_pool(name="ehat", bufs=3) as ehatp,
            tc.tile_pool(name="rec", bufs=1) as recp,
            tc.tile_pool(name="tmp", bufs=8) as tmpp,
            tc.tile_pool(name="tmph", bufs=8) as tmphp,
            tc.tile_pool(name="outz", bufs=4) as outzp,
            tc.tile_pool(name="pmm", bufs=2, space="PSUM") as pmm,
            tc.tile_pool(name="pacc", bufs=1, space="PSUM") as paccp,
            tc.tile_pool(name="pebc", bufs=2, space="PSUM") as pebcp,
        ):
            # ---- load constants ----
            # constants other than w1T/b1 go on the SWDGE queue so the sync
            # queue serves conv1's x/w chunks first (fast kernel start)
            def cload(name, dram, shape, dt=BF16, re="k p m -> p k m"):
                t = consts.tile(shape, dt, tag=name)
                nc.gpsimd.dma_start(out=t, in_=dram.rearrange(re) if re else dram)
                return t

            w1T = consts.tile([P, KC1, WIDTH], BF16, tag="w1T")
            b1 = consts.tile([P, PT, 1], F32, tag="b1")
            nc.sync.dma_start(out=b1, in_=b1_d.rearrange("k p m -> p k m"))
            wqT = cload("wqT", wqT_d, [P, PT, WIDTH])
            wkT = cload("wkT", wkT_d, [P, PT, WIDTH])
            wvT = cload("wvT", wvT_d, [P, PT, WIDTH])
            w3T = cload("w3T", w3T_d, [P, PT, OUT])
            bq = cload("bq", bq_d, [P, PT, 1], F32)
            bk = cload("bk", bk_d, [P, PT, 1], F32)
            bv = cload("bv", bv_d, [P, PT, 1], F32)
            batt = cload("batt", batt_d, [P, PT, 1], F32)
            b3 = cload("b3", b3_d, [P, OC, 1], F32)
            sel = cload("sel", sel_d, [P, PT, HEADS])
            p2 = cload("p2", p2_d, [P, PT, NT, P], re="k p m o -> p k m o")
            sab = cload("sab", sab_d, [P, HEADS], re=None)
            eye32 = cload("eye32", eye32_d, [HEADS, HEADS], re=None)
            ident = cload("ident", ident_d, [P, P], re=None)
            bm = cload("bm", bm_d, [P, PT, 4, P], re="m j r c -> r m j c")

            def head_bcast_ap(src16):
                # 2-level partition AP: dst[i*8+d] reads src partition (base+i)
                return bass.AP(tensor=src16.tensor, offset=src16.offset,
                               ap=[list(src16.ap[0]), [0, D]]
                                  + [list(a) for a in src16.ap[1:]])

            # persistent zero-padded k/v tiles, one pair per in-flight batch
            kpads, vpads = [], []
            for b in range(BL):
                kp = padp.tile([P, PT, H + 2, W + 2], BF16, tag="kpad")
                vp = padp.tile([P, PT, H + 2, W + 2], BF16, tag="vpad")
                nc.vector.memset(kp, 0.0)
                nc.vector.memset(vp, 0.0)
                kpads.append(kp)
                vpads.append(vp)

            # ---- per-batch state ----
            xbs = [None] * BL
            h1s = [None] * BL
            qs = [None] * BL
            epks = [[None] * NT for _ in range(BL)]
            h2s = [None] * BL

            def load_x(b):
                xb = xbp.tile([P, KC1, HW], BF16, tag="xb")
                xbs[b] = xb
                for kc in range(KC1):
                    if b == 0:
                        # separate HWDGE queue so w1T and x stream in parallel
                        nc.scalar.dma_start(out=w1T[:, kc, :], in_=w1T_d[kc])
                    nc.sync.dma_start(out=xb[:, kc, :], in_=x16_d[b, kc])

            def conv1_chunk(b, mc):
                xb = xbs[b]
                if mc == 0:
                    h1 = actp.tile([P, PT, HW], BF16, tag="h1")
                    h1s[b] = h1
                ps = pmm.tile([P, HW], F32, tag="mm")
                for kc in range(KC1):
                    for n in range(NHALF):
                        nc.tensor.matmul(
                            ps[:, _ns(n)],
                            w1T[:, kc, mc * P:(mc + 1) * P],
                            xb[:, kc, _ns(n)],
                            start=(kc == 0), stop=(kc == KC1 - 1),
                        )
                nc.scalar.activation(
                    out=h1s[b][:, mc, :], in_=ps,
                    func=mybir.ActivationFunctionType.Relu,
                    bias=b1[:, mc], scale=1.0,
                )

            def qkv_chunk(b, i):
                # i in 0..5: (q,k,v) x (mc0,mc1)
                h1 = h1s[b]
                if i == 0:
                    q = actp.tile([P, PT, HW], BF16, tag="q")
                    qs[b] = q
                specs = [(wqT, bq, True, None), (wkT, bk, True, kpads[b]),
                         (wvT, bv, False, vpads[b])]
                wT, bias, relu, dest = specs[i // 2]
                mc = i % 2
                ps = pmm.tile([P, HW], F32, tag="mm")
                for kc in range(PT):
                    for n in range(NHALF):
                        nc.tensor.matmul(
                            ps[:, _ns(n)],
                            wT[:, kc, mc * P:(mc + 1) * P],
                            h1[:, kc, _ns(n)],
                            start=(kc == 0), stop=(kc == PT - 1),
                        )
                if dest is None:
                    o, i_ = qs[b][:, mc, :], ps[:]
                else:
                    o = dest[:, mc, 1:H + 1, 1:W + 1]
                    i_ = ps.rearrange("p (a b) -> p a b", a=H)
                nc.scalar.activation(
                    out=o, in_=i_,
                    func=(mybir.ActivationFunctionType.Relu if relu
                          else mybir.ActivationFunctionType.Identity),
                    bias=bias[:, mc], scale=1.0,
                )

            def logits_tile(b, t):
                # packed tile t rows: 32*(kk%4) + g  for kk in {4t..4t+3}
                q, kpad = qs[b], kpads[b]
                nsh = 4 if t < 2 else 1
                rows = 32 * nsh
                Lpk = pmm.tile([P, HW], F32, tag="mm")
                # qpos term: all rows at once per pt chunk
                for n in range(NHALF):
                    for pt in range(PT):
                        nc.tensor.matmul(
                            Lpk[:rows, _ns(n)],
                            p2[:, pt, t, :rows],
                            q[:, pt, _ns(n)],
                            start=(pt == 0), stop=False,
                            skip_group_check=True,
                        )
                # qk products + col-tiled group reduce
                for j in range(nsh):
                    kk = 4 * t + j
                    di, dj = kk // KS, kk % KS
                    for pt in range(PT):
                        tmp = tmpp.tile([P, HW], BF16, tag="tmp")
                        nc.vector.tensor_tensor(
                            out=tmp.rearrange("p (a b) -> p a b", a=H),
                            in0=kpad[:, pt, di:di + H, dj:dj + W],
                            in1=q[:, pt, :].rearrange("p (a b) -> p a b", a=H),
                            op=mybir.AluOpType.mult,
                        )
                        for n in range(NHALF):
                            nc.tensor.matmul(
                                Lpk[32 * j:32 * (j + 1), _ns(n)],
                                sel[:, pt, :],
                                tmp[:, _ns(n)],
                                start=False, stop=(pt == PT - 1),
                                tile_position=(0, 32 * j),
                                skip_group_check=True,
                            )
                epk = epkp.tile([P, HW], BF16, tag="epk")
                nc.scalar.activation(
                    out=epk[:rows, :], in_=Lpk[:rows, :],
                    func=mybir.ActivationFunctionType.Exp,
                )
                epks[b][t] = epk

            ehats = [[None] * NT for _ in range(BL)]

            def softchain(b):
                # denominator (3 back-to-back PSUM-accumulated matmuls per
                # half), recip, recip cast+bcast to packed rows, ehat mults
                denp = pmm.tile([HEADS, HW], F32, tag="mm")
                for n in range(NHALF):
                    for t in range(NT):
                        rows = 128 if t < 2 else 32
                        lhs = sab if t < 2 else eye32
                        nc.tensor.matmul(
                            denp[:, _ns(n)], lhs[:rows, :],
                            epks[b][t][:rows, _ns(n)],
                            start=(t == 0), stop=(t == NT - 1),
                            skip_group_check=True,
                        )
                rec32 = recp.tile([HEADS, HW], F32, tag="rec32")
                nc.vector.reciprocal_approx_fast(out=rec32, in_=denp)
                recpk = recp.tile([P, HW], BF16, tag="recpk")
                for j in range(4):
                    # SWDGE: fp32 -> bf16 cast during DMA
                    nc.gpsimd.dma_start(out=recpk[32 * j:32 * (j + 1), :],
                                        in_=rec32)
                for t in range(NT):
                    rows = 128 if t < 2 else 32
                    ehat = ehatp.tile([P, HW], BF16, tag="ehat")
                    nc.vector.tensor_tensor(
                        out=ehat[:rows, :], in0=epks[b][t][:rows, :],
                        in1=recpk[:rows, :], op=mybir.AluOpType.mult,
                    )
                    ehats[b][t] = ehat

            def ev_start(b, mc):
                if mc == 0:
                    h2 = actp.tile([P, PT, HW], BF16, tag="h2")
                    h2s[b] = h2
                acc = paccp.tile([P, HW], F32, tag="acc")
                return acc

            def ev_kk(b, mc, acc, kk):
                # out_pre[c] += ehat_bc * v_shift for one shift kk
                # ehat broadcast head->channels via 0/1 matmul into PSUM halves
                vpad = vpads[b]
                HH = H // NHALF
                t, j = kk // 4, kk % 4
                di, dj = kk // KS, kk % KS
                rows = 128 if t < 2 else 32
                for n in range(NHALF):
                    eb = pebcp.tile([P, 512], F32, tag="eb")
                    nc.tensor.matmul(
                        eb, bm[:rows, mc, j, :], ehats[b][t][:rows, _ns(n)],
                        start=True, stop=True,
                        skip_group_check=True,
                    )
                    t2 = tmphp.tile([P, 512], BF16, tag="tmph")
                    a0 = di + HH * n
                    nc.vector.tensor_tensor(
                        out=t2.rearrange("p (a b) -> p a b", a=HH),
                        in0=eb.rearrange("p (a b) -> p a b", a=HH),
                        in1=vpad[:, mc, a0:a0 + HH, dj:dj + W],
                        op=mybir.AluOpType.mult,
                    )
                    nc.tensor.matmul(
                        acc[:, _ns(n)], ident, t2,
                        start=(kk == 0), stop=(kk == NKK - 1),
                        skip_group_check=True,
                    )

            def ev_end(b, mc, acc):
                nc.scalar.activation(
                    out=h2s[b][:, mc, :], in_=acc,
                    func=mybir.ActivationFunctionType.Relu,
                    bias=batt[:, mc], scale=1.0,
                )

            def conv3(b, ocs):
                h2, xb = h2s[b], xbs[b]
                for oc in ocs:
                    ps = pmm.tile([P, HW], F32, tag="mm")
                    for n in range(NHALF):
                        for kc in range(PT):
                            nc.tensor.matmul(
                                ps[:, _ns(n)],
                                w3T[:, kc, oc * P:(oc + 1) * P],
                                h2[:, kc, _ns(n)],
                                start=(kc == 0), stop=False,
                                skip_group_check=True,
                            )
                        nc.tensor.matmul(
                            ps[:, _ns(n)], ident, xb[:, oc, _ns(n)],
                            start=False, stop=True,
                            skip_group_check=True,
                        )
                    zr = outzp.tile([P, HW], BF16, tag="outzr")
                    nc.scalar.activation(
                        out=zr, in_=ps, func=mybir.ActivationFunctionType.Relu,
                        bias=b3[:, oc], scale=1.0,
                    )
                    eng = nc.scalar if oc % 2 == 0 else nc.sync
                    eng.dma_start(out=out_d[b, oc], in_=zr)

            # ---- pipelined schedule over the two batches ----
            # PE-dense conv chunks are interleaved into the DVE-bound
            # attention phases of the other batch (keeps PE fed and the HAM
            # clock warm).
            load_x(0)
            load_x(1)
            conv1_chunk(0, 0)
            conv1_chunk(0, 1)
            for i in range(6):
                qkv_chunk(0, i)
            logits_tile(0, 0)
            conv1_chunk(1, 0)
            logits_tile(0, 1)
            conv1_chunk(1, 1)
            logits_tile(0, 2)
            softchain(0)
            qkv_chunk(1, 0)
            qkv_chunk(1, 1)
            qkv_chunk(1, 2)
            acc = ev_start(0, 0)
            for kk in range(NKK):
                ev_kk(0, 0, acc, kk)
                if kk == 2:
                    qkv_chunk(1, 3)
                elif kk == 5:
                    qkv_chunk(1, 4)
                elif kk == 8:
                    qkv_chunk(1, 5)
            ev_end(0, 0, acc)
            logits_tile(1, 0)
            acc = ev_start(0, 1)
            for kk in range(4):
                ev_kk(0, 1, acc, kk)
            logits_tile(1, 1)
            for kk in range(4, NKK):
                ev_kk(0, 1, acc, kk)
            logits_tile(1, 2)
            ev_end(0, 1, acc)
            softchain(1)
            conv3(0, range(0, 5))
            acc = ev_start(1, 0)
            for kk in range(NKK):
                ev_kk(1, 0, acc, kk)
                if kk == 3:
                    conv3(0, [5])
                elif kk == 6:
                    conv3(0, [6])
            ev_end(1, 0, acc)
            acc = ev_start(1, 1)
            for kk in range(NKK):
                ev_kk(1, 1, acc, kk)
                if kk == 4:
                    conv3(0, [7])
            ev_end(1, 1, acc)
            conv3(1, range(OC))

    nc.compile()
    return nc


_PROG = None


def _host_prep(inputs):
    import ml_dtypes
    bf = ml_dtypes.bfloat16
    f = lambda a: np.asarray(a, dtype=np.float32)
    x = f(inputs["x"])
    # fold bn scales into weights (bn(conv(x,W),s,b) = conv(x, s*W) + b)
    w1 = f(inputs["w_conv1"]) * f(inputs["bn1_s"])[:, None]
    wq = f(inputs["wq"]) * f(inputs["bnq_s"])[:, None]
    wk = f(inputs["wk"]) * f(inputs["bnk_s"])[:, None]
    # fold bnatt scale through the (linear) attention-value path into v
    sv = f(inputs["bnatt_s"]) * f(inputs["bnv_s"])
    wv = f(inputs["wv"]) * sv[:, None]
    bv = f(inputs["bnatt_s"]) * f(inputs["bnv_b"])
    w3 = f(inputs["w_conv3"]) * f(inputs["bn3_s"])[:, None]

    posf = (f(inputs["pos_h"]) + f(inputs["pos_w"])).reshape(WIDTH, NKK)

    sel = np.zeros((PT, P, HEADS), np.float32)
    for pt in range(PT):
        for c in range(P):
            sel[pt, c, pt * (P // D) + c // D] = 1.0
    # p2[pt, c, t, 32*j+g] = pos[c_global, 4t+j] if head(c_global)==g
    p2 = np.zeros((PT, P, NT, P), np.float32)
    for pt in range(PT):
        for c in range(P):
            g = pt * (P // D) + c // D
            for kk in range(NKK):
                t, j = kk // 4, kk % 4
                p2[pt, c, t, 32 * j + g] = posf[pt * P + c, kk]
    # sab[r, g] = 1 if r % 32 == g (sum over the 4 packed kk rows)
    sab = np.zeros((P, HEADS), np.float32)
    for r in range(P):
        sab[r, r % HEADS] = 1.0
    # bm[mc, j, r, c] = 1 if r == 32*j + 16*mc + c//8 (head->channel bcast)
    bm = np.zeros((PT, 4, P, P), np.float32)
    for mc in range(PT):
        for j in range(4):
            for c in range(P):
                bm[mc, j, 32 * j + 16 * mc + c // D, c] = 1.0
    com = {
        "w1T": np.ascontiguousarray(w1.T.reshape(KC1, P, WIDTH)).astype(bf),
        "wqT": np.ascontiguousarray(wq.T.reshape(PT, P, WIDTH)).astype(bf),
        "wkT": np.ascontiguousarray(wk.T.reshape(PT, P, WIDTH)).astype(bf),
        "wvT": np.ascontiguousarray(wv.T.reshape(PT, P, WIDTH)).astype(bf),
        "w3T": np.ascontiguousarray(w3.T.reshape(PT, P, OUT)).astype(bf),
        "b1": f(inputs["bn1_b"]).reshape(PT, P, 1),
        "bq": f(inputs["bnq_b"]).reshape(PT, P, 1),
        "bk": f(inputs["bnk_b"]).reshape(PT, P, 1),
        "bv": bv.reshape(PT, P, 1),
        "batt": f(inputs["bnatt_b"]).reshape(PT, P, 1),
        "b3": f(inputs["bn3_b"]).reshape(OC, P, 1),
        "sel": sel.astype(bf),
        "p2": p2.astype(bf),
        "sab": sab.astype(bf),
        "eye32": np.eye(HEADS, dtype=np.float32).astype(bf),
        "ident": np.eye(P, dtype=np.float32).astype(bf),
        "bm": bm.astype(bf),
    }
    xr = x.reshape(B, KC1, P, HW)
    in_maps = []
    for c in range(NC_):
        xs = np.ascontiguousarray(xr[c * BL:(c + 1) * BL])
        in_maps.append(dict(com, x16=xs.astype(bf)))
    return in_maps


def kernel(**inputs):
    global _PROG
    if _PROG is None:
        _PROG = build_program()
    in_maps = _host_prep(inputs)
    res = run_bass_kernel_spmd(_PROG, in_maps, core_ids=list(range(NC_)))
    outs = [np.asarray(res.results[c]["out"], dtype=np.float32)
            .reshape(BL, OUT, H, W) for c in range(NC_)]
    return np.concatenate(outs, axis=0)


# revision 16
# speedup vs baseline: 1.3181x; 1.0324x over previous
"""Trainium2 Bass kernel for nn_Bottleneck_75325136437765 (sparse 3x3 local attention bottleneck).

Sharding: data-parallel over batch B=16 across 8 cores (2 batches/core), params replicated.

Per-core layout: channels on partitions, spatial (32*32=1024) on free dim. All matmuls bf16,
fp32 PSUM accumulation.

v2: software-pipelined over the 2 per-core batches so PE never idles past the HAM
re-throttle window; e normalized at head level (epk * recip broadcast) so the v-phase
consumes one big per-mc broadcast instead of 18 small ones; output stored bf16.

  conv1/qkv/conv3: plain matmuls (lhsT = transposed weights, host-precomputed, bn scales folded).
  attention logits, packed PSUM layout (row = 32*(kk%4) + head, 3 tiles of 4/4/1 shifts):
      L[g,kk,hw] = sum_d q[gd,hw]*k[gd,hw+off_kk]  (col-tiled 0/1-selection matmuls over products)
                 + sum_d q[gd,hw]*pos[gd,kk]       (P2 matmul, accumulated into same PSUM)
  softmax over kk without max-subtraction:
      e = exp(L) (packed, 3 ACT ops); den = sum_kk e via 0/1 matmuls; recip = 1/den
      e_hat = e * recip (packed, recip cast+broadcast to packed rows via SWDGE)
      e_hat rearranged to head-level [32, 9, HW], then ONE broadcast DMA per channel chunk
      out_pre[c,hw] = sum_kk e_hat_bc[c,kk,hw] * v[c,hw+off_kk]
        per-shift product on DVE; sum over kk via identity-matmul PSUM accumulation
      h2 = relu(out_pre + bnatt_b)   (ACT, straight from PSUM)
  residual: identity matmul on bf16 x accumulated into the conv3 PSUM group.
"""

import numpy as np

import concourse.bass as bass
import concourse.bacc as bacc
import concourse.tile as tile
from concourse import mybir
from concourse.bass_utils import run_bass_kernel_spmd

# ---- problem constants (hardcoded per contract) ----
B, CIN, H, W = 16, 1024, 32, 32
WIDTH, OUT, HEADS, KS = 256, 1024, 32, 3
D = WIDTH // HEADS            # 8 channels per head
HW = H * W                    # 1024
NC_ = 8                       # cores
BL = B // NC_                 # 2 batches per core
P = 128
KC1 = CIN // P                # 8 contraction chunks for conv1
PT = WIDTH // P               # 2 partition tiles for width-256 tensors
OC = OUT // P                 # 8 output ptiles for conv3
NKK = KS * KS                 # 9 shifts
NT = 3                        # packed logit tiles (4+4+1 shifts)
F32 = mybir.dt.float32
BF16 = mybir.dt.bfloat16
NHALF = 2                     # PSUM-bank limit: matmul N<=512 fp32 out


def _ns(n):
    return slice(n * 512, (n + 1) * 512)


def build_program():
    nc = bacc.Bacc(None, target_bir_lowering=False, debug=False)

    def din(name, shape, dt=BF16):
        return nc.dram_tensor(name, list(shape), dt, kind="ExternalInput").ap()

    x16_d = din("x16", (BL, KC1, P, HW))
    w1T_d = din("w1T", (KC1, P, WIDTH))
    wqT_d = din("wqT", (PT, P, WIDTH))
    wkT_d = din("wkT", (PT, P, WIDTH))
    wvT_d = din("wvT", (PT, P, WIDTH))
    w3T_d = din("w3T", (PT, P, OUT))
    b1_d = din("b1", (PT, P, 1), F32)
    bq_d = din("bq", (PT, P, 1), F32)
    bk_d = din("bk", (PT, P, 1), F32)
    bv_d = din("bv", (PT, P, 1), F32)
    batt_d = din("batt", (PT, P, 1), F32)
    b3_d = din("b3", (OC, P, 1), F32)
    sel_d = din("sel", (PT, P, HEADS))
    p2_d = din("p2", (PT, P, NT, P))
    sab_d = din("sab", (P, HEADS))
    eye32_d = din("eye32", (HEADS, HEADS))
    ident_d = din("ident", (P, P))
    bm_d = din("bm", (PT, 4, P, P))
    out_d = nc.dram_tensor("out", [BL, OC, P, HW], BF16, kind="ExternalOutput").ap()

    with tile.TileContext(nc) as tc:
        with (
            tc.tile_pool(name="consts", bufs=1) as consts,
            tc.tile_pool(name="xb", bufs=2) as xbp,
            tc.tile_pool(name="act", bufs=2) as actp,
            tc.tile_pool(name="pad", bufs=2) as padp,
            tc.tile_pool(name="epk", bufs=3) as epkp,
            tc.tile_pool(name="ehat", bufs=3) as ehatp,
            tc.tile_pool(name="rec", bufs=1) as recp,
            tc.tile_pool(name="tmp", bufs=8) as tmpp,
            tc.tile_pool(name="tmph", bufs=8) as tmphp,
            tc.tile_pool(name="outz", bufs=4) as outzp,
            tc.tile_pool(name="pmm", bufs=2, space="PSUM") as pmm,
            tc.tile_pool(name="pacc", bufs=1, space="PSUM") as paccp,
            tc.tile_pool(name="pebc", bufs=2, space="PSUM") as pebcp,
        ):
            # ---- load constants ----
            # constants other than w1T/b1 go on the SWDGE queue so the sync
            # queue serves conv1's x/w chunks first (fast kernel start)
            def cload(name, dram, shape, dt=BF16, re="k p m -> p k m"):
                t = consts.tile(shape, dt, tag=name)
                nc.gpsimd.dma_start(out=t, in_=dram.rearrange(re) if re else dram)
                return t

            w1T = consts.tile([P, KC1, WIDTH], BF16, tag="w1T")
            b1 = consts.tile([P, PT, 1], F32, tag="b1")
            nc.sync.dma_start(out=b1, in_=b1_d.rearrange("k p m -> p k m"))
            wqT = cload("wqT", wqT_d, [P, PT, WIDTH])
            wkT = cload("wkT", wkT_d, [P, PT, WIDTH])
            wvT = cload("wvT", wvT_d, [P, PT, WIDTH])
            w3T = cload("w3T", w3T_d, [P, PT, OUT])
            bq = cload("bq", bq_d, [P, PT, 1], F32)
            bk = cload("bk", bk_d, [P, PT, 1], F32)
            bv = cload("bv", bv_d, [P, PT, 1], F32)
            batt = cload("batt", batt_d, [P, PT, 1], F32)
            b3 = cload("b3", b3_d, [P, OC, 1], F32)
            sel = cload("sel", sel_d, [P, PT, HEADS])
            p2 = cload("p2", p2_d, [P, PT, NT, P], re="k p m o -> p k m o")
            sab = cload("sab", sab_d, [P, HEADS], re=None)
            eye32 = cload("eye32", eye32_d, [HEADS, HEADS], re=None)
            ident = cload("ident", ident_d, [P, P], re=None)
            bm = cload("bm", bm_d, [P, PT, 4, P], re="m j r c -> r m j c")

            def head_bcast_ap(src16):
                # 2-level partition AP: dst[i*8+d] reads src partition (base+i)
                return bass.AP(tensor=src16.tensor, offset=src16.offset,
                               ap=[list(src16.ap[0]), [0, D]]
                                  + [list(a) for a in src16.ap[1:]])

            # persistent zero-padded k/v tiles, one pair per in-flight batch
            kpads, vpads = [], []
            for b in range(BL):
                kp = padp.tile([P, PT, H + 2, W + 2], BF16, tag="kpad")
                vp = padp.tile([P, PT, H + 2, W + 2], BF16, tag="vpad")
                nc.vector.memset(kp, 0.0)
                nc.vector.memset(vp, 0.0)
                kpads.append(kp)
                vpads.append(vp)

            # ---- per-batch state ----
            xbs = [None] * BL
            h1s = [None] * BL
            qs = [None] * BL
            epks = [[None] * NT for _ in range(BL)]
            h2s = [None] * BL

            def load_x(b):
                xb = xbp.tile([P, KC1, HW], BF16, tag="xb")
                xbs[b] = xb
                for kc in range(KC1):
                    if b == 0:
                        # separate HWDGE queue so w1T and x stream in parallel
                        nc.scalar.dma_start(out=w1T[:, kc, :], in_=w1T_d[kc])
                    nc.sync.dma_start(out=xb[:, kc, :], in_=x16_d[b, kc])

            def conv1_chunk(b, mc):
                xb = xbs[b]
                if mc == 0:
                    h1 = actp.tile([P, PT, HW], BF16, tag="h1")
                    h1s[b] = h1
                ps = pmm.tile([P, HW], F32, tag="mm")
                for kc in range(KC1):
                    for n in range(NHALF):
                        nc.tensor.matmul(
                            ps[:, _ns(n)],
                            w1T[:, kc, mc * P:(mc + 1) * P],
                            xb[:, kc, _ns(n)],
                            start=(kc == 0), stop=(kc == KC1 - 1),
                        )
                nc.scalar.activation(
                    out=h1s[b][:, mc, :], in_=ps,
                    func=mybir.ActivationFunctionType.Relu,
                    bias=b1[:, mc], scale=1.0,
                )

            def qkv_chunk(b, i):
                # i in 0..5: (q,k,v) x (mc0,mc1)
                h1 = h1s[b]
                if i == 0:
                    q = actp.tile([P, PT, HW], BF16, tag="q")
                    qs[b] = q
                specs = [(wqT, bq, True, None), (wkT, bk, True, kpads[b]),
                         (wvT, bv, False, vpads[b])]
                wT, bias, relu, dest = specs[i // 2]
                mc = i % 2
                ps = pmm.tile([P, HW], F32, tag="mm")
                for kc in range(PT):
                    for n in range(NHALF):
                        nc.tensor.matmul(
                            ps[:, _ns(n)],
                            wT[:, kc, mc * P:(mc + 1) * P],
                            h1[:, kc, _ns(n)],
                            start=(kc == 0), stop=(kc == PT - 1),
                        )
                if dest is None:
                    o, i_ = qs[b][:, mc, :], ps[:]
                else:
                    o = dest[:, mc, 1:H + 1, 1:W + 1]
                    i_ = ps.rearrange("p (a b) -> p a b", a=H)
                nc.scalar.activation(
                    out=o, in_=i_,
                    func=(mybir.ActivationFunctionType.Relu if relu
                          else mybir.ActivationFunctionType.Identity),
                    bias=bias[:, mc], scale=1.0,
                )

            def _qpos(b, t, Lpk):
                rows = 128 if t < 2 else 32
                for n in range(NHALF):
                    for pt in range(PT):
                        nc.tensor.matmul(
                            Lpk[:rows, _ns(n)],
                            p2[:, pt, t, :rows],
                            qs[b][:, pt, _ns(n)],
                            start=(pt == 0), stop=False,
                            skip_group_check=True,
                        )

            def _select(b, kk, prod3s, Lpks):
                t, j = kk // 4, kk % 4
                dj = kk % KS
                for pt in range(PT):
                    for n in range(NHALF):
                        nc.tensor.matmul(
                            Lpks[t][32 * j:32 * (j + 1), _ns(n)],
                            sel[:, pt, :],
                            prod3s[pt][:, dj, _ns(n)],
                            start=False, stop=(pt == PT - 1),
                            tile_position=(0, 32 * j),
                            skip_group_check=True,
                        )

            def _exp(b, t, Lpks):
                rows = 128 if t < 2 else 32
                epk = epkp.tile([P, HW], BF16, tag="epk")
                nc.scalar.activation(
                    out=epk[:rows, :], in_=Lpks[t][:rows, :],
                    func=mybir.ActivationFunctionType.Exp,
                )
                epks[b][t] = epk

            def logits_all(b, fillers=None):
                # packed tile t rows: 32*(kk%4) + g for kk in {4t..4t+3};
                # qk products grouped 3 shifts (fixed di) per DVE op
                q, kpad = qs[b], kpads[b]
                Lpks = {}
                for di in range(KS):
                    prod3s = []
                    for pt in range(PT):
                        prod3 = tmpp.tile([P, KS, HW], BF16, tag="tmp")
                        w = kpad[:, pt, di:di + H, 0:W]
                        in0 = bass.AP(tensor=w.tensor, offset=w.offset,
                                      ap=[list(w.ap[0]), [1, KS],
                                          [H + 2, H], [1, W]])
                        qv = q[:, pt, :]
                        in1 = bass.AP(tensor=qv.tensor, offset=qv.offset,
                                      ap=[list(qv.ap[0]), [0, KS],
                                          [W, H], [1, W]])
                        nc.vector.tensor_tensor(
                            out=prod3.rearrange("p k (a b) -> p k a b", a=H),
                            in0=in0, in1=in1, op=mybir.AluOpType.mult,
                        )
                        prod3s.append(prod3)
                    if di == 0:
                        Lpk0 = pmm.tile([P, HW], F32, tag="mm")
                        Lpks[0] = Lpk0
                        _qpos(b, 0, Lpk0)
                        for dj in range(KS):
                            _select(b, dj, prod3s, Lpks)
                    elif di == 1:
                        _select(b, 3, prod3s, Lpks)
                        _exp(b, 0, Lpks)
                        Lpk1 = pmm.tile([P, HW], F32, tag="mm")
                        Lpks[1] = Lpk1
                        _qpos(b, 1, Lpk1)
                        _select(b, 4, prod3s, Lpks)
                        _select(b, 5, prod3s, Lpks)
                    else:
                        _select(b, 6, prod3s, Lpks)
                        _select(b, 7, prod3s, Lpks)
                        _exp(b, 1, Lpks)
                        Lpk2 = pmm.tile([P, HW], F32, tag="mm")
                        Lpks[2] = Lpk2
                        _qpos(b, 2, Lpk2)
                        _select(b, 8, prod3s, Lpks)
                        _exp(b, 2, Lpks)
                    if fillers and di in fillers:
                        fillers[di]()

            ehats = [[None] * NT for _ in range(BL)]

            def softchain(b):
                # denominator (3 back-to-back PSUM-accumulated matmuls per
                # half), recip, recip cast+bcast to packed rows, ehat mults
                denp = pmm.tile([HEADS, HW], F32, tag="mm")
                for n in range(NHALF):
                    for t in range(NT):
                        rows = 128 if t < 2 else 32
                        lhs = sab if t < 2 else eye32
                        nc.tensor.matmul(
                            denp[:, _ns(n)], lhs[:rows, :],
                            epks[b][t][:rows, _ns(n)],
                            start=(t == 0), stop=(t == NT - 1),
                            skip_group_check=True,
                        )
                rec32 = recp.tile([HEADS, HW], F32, tag="rec32")
                nc.vector.reciprocal_approx_fast(out=rec32, in_=denp)
                recpk = recp.tile([P, HW], BF16, tag="recpk")
                for j in range(4):
                    # SWDGE: fp32 -> bf16 cast during DMA
                    nc.gpsimd.dma_start(out=recpk[32 * j:32 * (j + 1), :],
                                        in_=rec32)
                for t in range(NT):
                    rows = 128 if t < 2 else 32
                    ehat = ehatp.tile([P, HW], BF16, tag="ehat")
                    nc.vector.tensor_tensor(
                        out=ehat[:rows, :], in0=epks[b][t][:rows, :],
                        in1=recpk[:rows, :], op=mybir.AluOpType.mult,
                    )
                    ehats[b][t] = ehat

            def ev_start(b, mc):
                if mc == 0:
                    h2 = actp.tile([P, PT, HW], BF16, tag="h2")
                    h2s[b] = h2
                acc = paccp.tile([P, HW], F32, tag="acc")
                return acc

            def ev_kk(b, mc, acc, kk):
                # out_pre[c] += ehat_bc * v_shift for one shift kk
                # ehat broadcast head->channels via 0/1 matmul into PSUM halves
                vpad = vpads[b]
                HH = H // NHALF
                t, j = kk // 4, kk % 4
                di, dj = kk // KS, kk % KS
                rows = 128 if t < 2 else 32
                for n in range(NHALF):
                    eb = pebcp.tile([P, 512], F32, tag="eb")
                    nc.tensor.matmul(
                        eb, bm[:rows, mc, j, :], ehats[b][t][:rows, _ns(n)],
                        start=True, stop=True,
                        skip_group_check=True,
                    )
                    t2 = tmphp.tile([P, 512], BF16, tag="tmph")
                    a0 = di + HH * n
                    nc.vector.tensor_tensor(
                        out=t2.rearrange("p (a b) -> p a b", a=HH),
                        in0=eb.rearrange("p (a b) -> p a b", a=HH),
                        in1=vpad[:, mc, a0:a0 + HH, dj:dj + W],
                        op=mybir.AluOpType.mult,
                    )
                    nc.tensor.matmul(
                        acc[:, _ns(n)], ident, t2,
                        start=(kk == 0), stop=(kk == NKK - 1),
                        skip_group_check=True,
                    )

            def ev_end(b, mc, acc):
                nc.scalar.activation(
                    out=h2s[b][:, mc, :], in_=acc,
                    func=mybir.ActivationFunctionType.Relu,
                    bias=batt[:, mc], scale=1.0,
                )

            def conv3(b, ocs):
                h2, xb = h2s[b], xbs[b]
                for oc in ocs:
                    ps = pmm.tile([P, HW], F32, tag="mm")
                    for n in range(NHALF):
                        for kc in range(PT):
                            nc.tensor.matmul(
                                ps[:, _ns(n)],
                                w3T[:, kc, oc * P:(oc + 1) * P],
                                h2[:, kc, _ns(n)],
                                start=(kc == 0), stop=False,
                                skip_group_check=True,
                            )
                        nc.tensor.matmul(
                            ps[:, _ns(n)], ident, xb[:, oc, _ns(n)],
                            start=False, stop=True,
                            skip_group_check=True,
                        )
                    zr = outzp.tile([P, HW], BF16, tag="outzr")
                    nc.scalar.activation(
                        out=zr, in_=ps, func=mybir.ActivationFunctionType.Relu,
                        bias=b3[:, oc], scale=1.0,
                    )
                    eng = nc.scalar if oc % 2 == 0 else nc.sync
                    eng.dma_start(out=out_d[b, oc], in_=zr)

            # ---- pipelined schedule over the two batches ----
            # PE-dense conv chunks are interleaved into the DVE-bound
            # attention phases of the other batch (keeps PE fed and the HAM
            # clock warm).
            load_x(0)
            load_x(1)
            conv1_chunk(0, 0)
            conv1_chunk(0, 1)
            for i in range(6):
                qkv_chunk(0, i)
            logits_all(0, {0: lambda: conv1_chunk(1, 0),
                           1: lambda: conv1_chunk(1, 1)})
            softchain(0)
            qkv_chunk(1, 0)
            qkv_chunk(1, 1)
            qkv_chunk(1, 2)
            acc = ev_start(0, 0)
            for kk in range(NKK):
                ev_kk(0, 0, acc, kk)
                if kk == 2:
                    qkv_chunk(1, 3)
                elif kk == 5:
                    qkv_chunk(1, 4)
                elif kk == 8:
                    qkv_chunk(1, 5)
            ev_end(0, 0, acc)
            acc = ev_start(0, 1)
            logits_all(1, {0: lambda: [ev_kk(0, 1, acc, kk)
                                       for kk in range(4)],
                           1: lambda: [ev_kk(0, 1, acc, kk)
                                       for kk in range(4, NKK)]})
            ev_end(0, 1, acc)
            softchain(1)
            conv3(0, range(0, 5))
            acc = ev_start(1, 0)
            for kk in range(NKK):
                ev_kk(1, 0, acc, kk)
                if kk == 3:
                    conv3(0, [5])
                elif kk == 6:
                    conv3(0, [6])
            ev_end(1, 0, acc)
            acc = ev_start(1, 1)
            for kk in range(NKK):
                ev_kk(1, 1, acc, kk)
                if kk == 4:
                    conv3(0, [7])
            ev_end(1, 1, acc)
            conv3(1, range(OC))

    nc.compile()
    return nc


_PROG = None


def _host_prep(inputs):
    import ml_dtypes
    bf = ml_dtypes.bfloat16
    f = lambda a: np.asarray(a, dtype=np.float32)
    x = f(inputs["x"])
    # fold bn scales into weights (bn(conv(x,W),s,b) = conv(x, s*W) + b)
    w1 = f(inputs["w_conv1"]) * f(inputs["bn1_s"])[:, None]
    wq = f(inputs["wq"]) * f(inputs["bnq_s"])[:, None]
    wk = f(inputs["wk"]) * f(inputs["bnk_s"])[:, None]
    # fold bnatt scale through the (linear) attention-value path into v
    sv = f(inputs["bnatt_s"]) * f(inputs["bnv_s"])
    wv = f(inputs["wv"]) * sv[:, None]
    bv = f(inputs["bnatt_s"]) * f(inputs["bnv_b"])
    w3 = f(inputs["w_conv3"]) * f(inputs["bn3_s"])[:, None]

    posf = (f(inputs["pos_h"]) + f(inputs["pos_w"])).reshape(WIDTH, NKK)

    sel = np.zeros((PT, P, HEADS), np.float32)
    for pt in range(PT):
        for c in range(P):
            sel[pt, c, pt * (P // D) + c // D] = 1.0
    # p2[pt, c, t, 32*j+g] = pos[c_global, 4t+j] if head(c_global)==g
    p2 = np.zeros((PT, P, NT, P), np.float32)
    for pt in range(PT):
        for c in range(P):
            g = pt * (P // D) + c // D
            for kk in range(NKK):
                t, j = kk // 4, kk % 4
                p2[pt, c, t, 32 * j + g] = posf[pt * P + c, kk]
    # sab[r, g] = 1 if r % 32 == g (sum over the 4 packed kk rows)
    sab = np.zeros((P, HEADS), np.float32)
    for r in range(P):
        sab[r, r % HEADS] = 1.0
    # bm[mc, j, r, c] = 1 if r == 32*j + 16*mc + c//8 (head->channel bcast)
    bm = np.zeros((PT, 4, P, P), np.float32)
    for mc in range(PT):
        for j in range(4):
            for c in range(P):
                bm[mc, j, 32 * j + 16 * mc + c // D, c] = 1.0
    com = {
        "w1T": np.ascontiguousarray(w1.T.reshape(KC1, P, WIDTH)).astype(bf),
        "wqT": np.ascontiguousarray(wq.T.reshape(PT, P, WIDTH)).astype(bf),
        "wkT": np.ascontiguousarray(wk.T.reshape(PT, P, WIDTH)).astype(bf),
        "wvT": np.ascontiguousarray(wv.T.reshape(PT, P, WIDTH)).astype(bf),
        "w3T": np.ascontiguousarray(w3.T.reshape(PT, P, OUT)).astype(bf),
        "b1": f(inputs["bn1_b"]).reshape(PT, P, 1),
        "bq": f(inputs["bnq_b"]).reshape(PT, P, 1),
        "bk": f(inputs["bnk_b"]).reshape(PT, P, 1),
        "bv": bv.reshape(PT, P, 1),
        "batt": f(inputs["bnatt_b"]).reshape(PT, P, 1),
        "b3": f(inputs["bn3_b"]).reshape(OC, P, 1),
        "sel": sel.astype(bf),
        "p2": p2.astype(bf),
        "sab": sab.astype(bf),
        "eye32": np.eye(HEADS, dtype=np.float32).astype(bf),
        "ident": np.eye(P, dtype=np.float32).astype(bf),
        "bm": bm.astype(bf),
    }
    xr = x.reshape(B, KC1, P, HW)
    in_maps = []
    for c in range(NC_):
        xs = np.ascontiguousarray(xr[c * BL:(c + 1) * BL])
        in_maps.append(dict(com, x16=xs.astype(bf)))
    return in_maps


def kernel(**inputs):
    global _PROG
    if _PROG is None:
        _PROG = build_program()
    in_maps = _host_prep(inputs)
    res = run_bass_kernel_spmd(_PROG, in_maps, core_ids=list(range(NC_)))
    outs = [np.asarray(res.results[c]["out"], dtype=np.float32)
            .reshape(BL, OUT, H, W) for c in range(NC_)]
    return np.concatenate(outs, axis=0)


# revision 17
# speedup vs baseline: 1.3191x; 1.0008x over previous
"""Trainium2 Bass kernel for nn_Bottleneck_75325136437765 (sparse 3x3 local attention bottleneck).

Sharding: data-parallel over batch B=16 across 8 cores (2 batches/core), params replicated.

Per-core layout: channels on partitions, spatial (32*32=1024) on free dim. All matmuls bf16,
fp32 PSUM accumulation.

v2: software-pipelined over the 2 per-core batches so PE never idles past the HAM
re-throttle window; e normalized at head level (epk * recip broadcast) so the v-phase
consumes one big per-mc broadcast instead of 18 small ones; output stored bf16.

  conv1/qkv/conv3: plain matmuls (lhsT = transposed weights, host-precomputed, bn scales folded).
  attention logits, packed PSUM layout (row = 32*(kk%4) + head, 3 tiles of 4/4/1 shifts):
      L[g,kk,hw] = sum_d q[gd,hw]*k[gd,hw+off_kk]  (col-tiled 0/1-selection matmuls over products)
                 + sum_d q[gd,hw]*pos[gd,kk]       (P2 matmul, accumulated into same PSUM)
  softmax over kk without max-subtraction:
      e = exp(L) (packed, 3 ACT ops); den = sum_kk e via 0/1 matmuls; recip = 1/den
      e_hat = e * recip (packed, recip cast+broadcast to packed rows via SWDGE)
      e_hat rearranged to head-level [32, 9, HW], then ONE broadcast DMA per channel chunk
      out_pre[c,hw] = sum_kk e_hat_bc[c,kk,hw] * v[c,hw+off_kk]
        per-shift product on DVE; sum over kk via identity-matmul PSUM accumulation
      h2 = relu(out_pre + bnatt_b)   (ACT, straight from PSUM)
  residual: identity matmul on bf16 x accumulated into the conv3 PSUM group.
"""

import numpy as np

import concourse.bass as bass
import concourse.bacc as bacc
import concourse.tile as tile
from concourse import mybir
from concourse.bass_utils import run_bass_kernel_spmd

# ---- problem constants (hardcoded per contract) ----
B, CIN, H, W = 16, 1024, 32, 32
WIDTH, OUT, HEADS, KS = 256, 1024, 32, 3
D = WIDTH // HEADS            # 8 channels per head
HW = H * W                    # 1024
NC_ = 8                       # cores
BL = B // NC_                 # 2 batches per core
P = 128
KC1 = CIN // P                # 8 contraction chunks for conv1
PT = WIDTH // P               # 2 partition tiles for width-256 tensors
OC = OUT // P                 # 8 output ptiles for conv3
NKK = KS * KS                 # 9 shifts
NT = 3                        # packed logit tiles (4+4+1 shifts)
F32 = mybir.dt.float32
BF16 = mybir.dt.bfloat16
NHALF = 2                     # PSUM-bank limit: matmul N<=512 fp32 out


def _ns(n):
    return slice(n * 512, (n + 1) * 512)


def build_program():
    nc = bacc.Bacc(None, target_bir_lowering=False, debug=False)

    def din(name, shape, dt=BF16):
        return nc.dram_tensor(name, list(shape), dt, kind="ExternalInput").ap()

    x16_d = din("x16", (BL, KC1, P, HW))
    w1T_d = din("w1T", (KC1, P, WIDTH))
    wqT_d = din("wqT", (PT, P, WIDTH))
    wkT_d = din("wkT", (PT, P, WIDTH))
    wvT_d = din("wvT", (PT, P, WIDTH))
    w3T_d = din("w3T", (PT, P, OUT))
    b1_d = din("b1", (PT, P, 1), F32)
    bq_d = din("bq", (PT, P, 1), F32)
    bk_d = din("bk", (PT, P, 1), F32)
    bv_d = din("bv", (PT, P, 1), F32)
    batt_d = din("batt", (PT, P, 1), F32)
    b3_d = din("b3", (OC, P, 1), F32)
    sel_d = din("sel", (PT, P, HEADS))
    p2_d = din("p2", (PT, P, NT, P))
    sab_d = din("sab", (P, HEADS))
    eye32_d = din("eye32", (HEADS, HEADS))
    ident_d = din("ident", (P, P))
    bm_d = din("bm", (PT, 4, P, P))
    out_d = nc.dram_tensor("out", [BL, OC, P, HW], BF16, kind="ExternalOutput").ap()

    with tile.TileContext(nc) as tc:
        with (
            tc.tile_pool(name="consts", bufs=1) as consts,
            tc.tile_pool(name="xb", bufs=2) as xbp,
            tc.tile_pool(name="act", bufs=2) as actp,
            tc.tile_pool(name="pad", bufs=2) as padp,
            tc.tile_pool(name="epk", bufs=3) as epkp,
            tc.tile_pool(name="ehat", bufs=3) as ehatp,
            tc.tile_pool(name="rec", bufs=1) as recp,
            tc.tile_pool(name="tmp", bufs=8) as tmpp,
            tc.tile_pool(name="tmph", bufs=8) as tmphp,
            tc.tile_pool(name="outz", bufs=4) as outzp,
            tc.tile_pool(name="pmm", bufs=2, space="PSUM") as pmm,
            tc.tile_pool(name="pacc", bufs=1, space="PSUM") as paccp,
            tc.tile_pool(name="pebc", bufs=2, space="PSUM") as pebcp,
        ):
            # ---- load constants ----
            # constants other than w1T/b1 go on the SWDGE queue so the sync
            # queue serves conv1's x/w chunks first (fast kernel start)
            def cload(name, dram, shape, dt=BF16, re="k p m -> p k m"):
                t = consts.tile(shape, dt, tag=name)
                nc.gpsimd.dma_start(out=t, in_=dram.rearrange(re) if re else dram)
                return t

            w1T = consts.tile([P, KC1, WIDTH], BF16, tag="w1T")
            b1 = consts.tile([P, PT, 1], F32, tag="b1")
            nc.sync.dma_start(out=b1, in_=b1_d.rearrange("k p m -> p k m"))
            wqT = cload("wqT", wqT_d, [P, PT, WIDTH])
            wkT = cload("wkT", wkT_d, [P, PT, WIDTH])
            wvT = cload("wvT", wvT_d, [P, PT, WIDTH])
            w3T = cload("w3T", w3T_d, [P, PT, OUT])
            bq = cload("bq", bq_d, [P, PT, 1], F32)
            bk = cload("bk", bk_d, [P, PT, 1], F32)
            bv = cload("bv", bv_d, [P, PT, 1], F32)
            batt = cload("batt", batt_d, [P, PT, 1], F32)
            b3 = cload("b3", b3_d, [P, OC, 1], F32)
            sel = cload("sel", sel_d, [P, PT, HEADS])
            p2 = cload("p2", p2_d, [P, PT, NT, P], re="k p m o -> p k m o")
            sab = cload("sab", sab_d, [P, HEADS], re=None)
            eye32 = cload("eye32", eye32_d, [HEADS, HEADS], re=None)
            ident = consts.tile([P, P], BF16, tag="ident")
            nc.sync.dma_start(out=ident, in_=ident_d)
            bm = cload("bm", bm_d, [P, PT, 4, P], re="m j r c -> r m j c")

            def head_bcast_ap(src16):
                # 2-level partition AP: dst[i*8+d] reads src partition (base+i)
                return bass.AP(tensor=src16.tensor, offset=src16.offset,
                               ap=[list(src16.ap[0]), [0, D]]
                                  + [list(a) for a in src16.ap[1:]])

            # persistent zero-padded k/v tiles, one pair per in-flight batch
            kpads, vpads = [], []
            for b in range(BL):
                kp = padp.tile([P, PT, H + 2, W + 2], BF16, tag="kpad")
                vp = padp.tile([P, PT, H + 2, W + 2], BF16, tag="vpad")
                nc.vector.memset(kp, 0.0)
                nc.vector.memset(vp, 0.0)
                kpads.append(kp)
                vpads.append(vp)

            # ---- per-batch state ----
            xbs = [None] * BL
            h1s = [None] * BL
            qs = [None] * BL
            epks = [[None] * NT for _ in range(BL)]
            h2s = [None] * BL

            def load_x(b):
                xb = xbp.tile([P, KC1, HW], BF16, tag="xb")
                xbs[b] = xb
                for kc in range(KC1):
                    if b == 0:
                        # separate HWDGE queue so w1T and x stream in parallel
                        nc.scalar.dma_start(out=w1T[:, kc, :], in_=w1T_d[kc])
                    nc.sync.dma_start(out=xb[:, kc, :], in_=x16_d[b, kc])

            def conv1_chunk(b, mc):
                xb = xbs[b]
                if mc == 0:
                    h1 = actp.tile([P, PT, HW], BF16, tag="h1")
                    h1s[b] = h1
                ps = pmm.tile([P, HW], F32, tag="mm")
                for kc in range(KC1):
                    for n in range(NHALF):
                        nc.tensor.matmul(
                            ps[:, _ns(n)],
                            w1T[:, kc, mc * P:(mc + 1) * P],
                            xb[:, kc, _ns(n)],
                            start=(kc == 0), stop=(kc == KC1 - 1),
                        )
                nc.scalar.activation(
                    out=h1s[b][:, mc, :], in_=ps,
                    func=mybir.ActivationFunctionType.Relu,
                    bias=b1[:, mc], scale=1.0,
                )

            def qkv_chunk(b, i):
                # i in 0..5: (q,k,v) x (mc0,mc1)
                h1 = h1s[b]
                if i == 0:
                    q = actp.tile([P, PT, HW], BF16, tag="q")
                    qs[b] = q
                specs = [(wqT, bq, True, None), (wkT, bk, True, kpads[b]),
                         (wvT, bv, False, vpads[b])]
                wT, bias, relu, dest = specs[i // 2]
                mc = i % 2
                ps = pmm.tile([P, HW], F32, tag="mm")
                for kc in range(PT):
                    for n in range(NHALF):
                        nc.tensor.matmul(
                            ps[:, _ns(n)],
                            wT[:, kc, mc * P:(mc + 1) * P],
                            h1[:, kc, _ns(n)],
                            start=(kc == 0), stop=(kc == PT - 1),
                        )
                if dest is None:
                    o, i_ = qs[b][:, mc, :], ps[:]
                else:
                    o = dest[:, mc, 1:H + 1, 1:W + 1]
                    i_ = ps.rearrange("p (a b) -> p a b", a=H)
                nc.scalar.activation(
                    out=o, in_=i_,
                    func=(mybir.ActivationFunctionType.Relu if relu
                          else mybir.ActivationFunctionType.Identity),
                    bias=bias[:, mc], scale=1.0,
                )

            def _qpos(b, t, Lpk):
                rows = 128 if t < 2 else 32
                for n in range(NHALF):
                    for pt in range(PT):
                        nc.tensor.matmul(
                            Lpk[:rows, _ns(n)],
                            p2[:, pt, t, :rows],
                            qs[b][:, pt, _ns(n)],
                            start=(pt == 0), stop=False,
                            skip_group_check=True,
                        )

            def _select(b, kk, prod3s, Lpks):
                t, j = kk // 4, kk % 4
                dj = kk % KS
                for pt in range(PT):
                    for n in range(NHALF):
                        nc.tensor.matmul(
                            Lpks[t][32 * j:32 * (j + 1), _ns(n)],
                            sel[:, pt, :],
                            prod3s[pt][:, dj, _ns(n)],
                            start=False, stop=(pt == PT - 1),
                            tile_position=(0, 32 * j),
                            skip_group_check=True,
                        )

            def _exp(b, t, Lpks):
                rows = 128 if t < 2 else 32
                epk = epkp.tile([P, HW], BF16, tag="epk")
                nc.scalar.activation(
                    out=epk[:rows, :], in_=Lpks[t][:rows, :],
                    func=mybir.ActivationFunctionType.Exp,
                )
                epks[b][t] = epk

            def logits_all(b, fillers=None):
                # packed tile t rows: 32*(kk%4) + g for kk in {4t..4t+3};
                # qk products grouped 3 shifts (fixed di) per DVE op
                q, kpad = qs[b], kpads[b]
                Lpks = {}
                for di in range(KS):
                    prod3s = []
                    for pt in range(PT):
                        prod3 = tmpp.tile([P, KS, HW], BF16, tag="tmp")
                        w = kpad[:, pt, di:di + H, 0:W]
                        in0 = bass.AP(tensor=w.tensor, offset=w.offset,
                                      ap=[list(w.ap[0]), [1, KS],
                                          [H + 2, H], [1, W]])
                        qv = q[:, pt, :]
                        in1 = bass.AP(tensor=qv.tensor, offset=qv.offset,
                                      ap=[list(qv.ap[0]), [0, KS],
                                          [W, H], [1, W]])
                        nc.vector.tensor_tensor(
                            out=prod3.rearrange("p k (a b) -> p k a b", a=H),
                            in0=in0, in1=in1, op=mybir.AluOpType.mult,
                        )
                        prod3s.append(prod3)
                    if di == 0:
                        Lpk0 = pmm.tile([P, HW], F32, tag="mm")
                        Lpks[0] = Lpk0
                        _qpos(b, 0, Lpk0)
                        for dj in range(KS):
                            _select(b, dj, prod3s, Lpks)
                    elif di == 1:
                        _select(b, 3, prod3s, Lpks)
                        _exp(b, 0, Lpks)
                        Lpk1 = pmm.tile([P, HW], F32, tag="mm")
                        Lpks[1] = Lpk1
                        _qpos(b, 1, Lpk1)
                        _select(b, 4, prod3s, Lpks)
                        _select(b, 5, prod3s, Lpks)
                    else:
                        _select(b, 6, prod3s, Lpks)
                        _select(b, 7, prod3s, Lpks)
                        _exp(b, 1, Lpks)
                        Lpk2 = pmm.tile([P, HW], F32, tag="mm")
                        Lpks[2] = Lpk2
                        _qpos(b, 2, Lpk2)
                        _select(b, 8, prod3s, Lpks)
                        _exp(b, 2, Lpks)
                    if fillers and di in fillers:
                        fillers[di]()

            ehats = [[None] * NT for _ in range(BL)]

            def softchain(b):
                # denominator (3 back-to-back PSUM-accumulated matmuls per
                # half), recip, recip cast+bcast to packed rows, ehat mults
                denp = pmm.tile([HEADS, HW], F32, tag="mm")
                for n in range(NHALF):
                    for t in range(NT):
                        rows = 128 if t < 2 else 32
                        lhs = sab if t < 2 else eye32
                        nc.tensor.matmul(
                            denp[:, _ns(n)], lhs[:rows, :],
                            epks[b][t][:rows, _ns(n)],
                            start=(t == 0), stop=(t == NT - 1),
                            skip_group_check=True,
                        )
                rec32 = recp.tile([HEADS, HW], F32, tag="rec32")
                nc.vector.reciprocal_approx_fast(out=rec32, in_=denp)
                recpk = recp.tile([P, HW], BF16, tag="recpk")
                for j in range(4):
                    # SWDGE: fp32 -> bf16 cast during DMA
                    nc.gpsimd.dma_start(out=recpk[32 * j:32 * (j + 1), :],
                                        in_=rec32)
                for t in range(NT):
                    rows = 128 if t < 2 else 32
                    ehat = ehatp.tile([P, HW], BF16, tag="ehat")
                    nc.vector.tensor_tensor(
                        out=ehat[:rows, :], in0=epks[b][t][:rows, :],
                        in1=recpk[:rows, :], op=mybir.AluOpType.mult,
                    )
                    ehats[b][t] = ehat

            def ev_start(b, mc):
                if mc == 0:
                    h2 = actp.tile([P, PT, HW], BF16, tag="h2")
                    h2s[b] = h2
                acc = paccp.tile([P, HW], F32, tag="acc")
                return acc

            def ev_kk(b, mc, acc, kk):
                # out_pre[c] += ehat_bc * v_shift for one shift kk
                # ehat broadcast head->channels via 0/1 matmul into PSUM halves
                vpad = vpads[b]
                HH = H // NHALF
                t, j = kk // 4, kk % 4
                di, dj = kk // KS, kk % KS
                rows = 128 if t < 2 else 32
                for n in range(NHALF):
                    eb = pebcp.tile([P, 512], F32, tag="eb")
                    nc.tensor.matmul(
                        eb, bm[:rows, mc, j, :], ehats[b][t][:rows, _ns(n)],
                        start=True, stop=True,
                        skip_group_check=True,
                    )
                    t2 = tmphp.tile([P, 512], BF16, tag="tmph")
                    a0 = di + HH * n
                    nc.vector.tensor_tensor(
                        out=t2.rearrange("p (a b) -> p a b", a=HH),
                        in0=eb.rearrange("p (a b) -> p a b", a=HH),
                        in1=vpad[:, mc, a0:a0 + HH, dj:dj + W],
                        op=mybir.AluOpType.mult,
                    )
                    nc.tensor.matmul(
                        acc[:, _ns(n)], ident, t2,
                        start=(kk == 0), stop=(kk == NKK - 1),
                        skip_group_check=True,
                    )

            def ev_end(b, mc, acc):
                nc.scalar.activation(
                    out=h2s[b][:, mc, :], in_=acc,
                    func=mybir.ActivationFunctionType.Relu,
                    bias=batt[:, mc], scale=1.0,
                )

            def conv3(b, ocs):
                h2, xb = h2s[b], xbs[b]
                for oc in ocs:
                    ps = pmm.tile([P, HW], F32, tag="mm")
                    for n in range(NHALF):
                        for kc in range(PT):
                            nc.tensor.matmul(
                                ps[:, _ns(n)],
                                w3T[:, kc, oc * P:(oc + 1) * P],
                                h2[:, kc, _ns(n)],
                                start=(kc == 0), stop=False,
                                skip_group_check=True,
                            )
                        nc.tensor.matmul(
                            ps[:, _ns(n)], ident, xb[:, oc, _ns(n)],
                            start=False, stop=True,
                            skip_group_check=True,
                        )
                    zr = outzp.tile([P, HW], BF16, tag="outzr")
                    nc.scalar.activation(
                        out=zr, in_=ps, func=mybir.ActivationFunctionType.Relu,
                        bias=b3[:, oc], scale=1.0,
                    )
                    eng = nc.scalar if oc % 2 == 0 else nc.sync
                    eng.dma_start(out=out_d[b, oc], in_=zr)

            # ---- HAM warm-up: PE busy through one full activity window ----
            wrhs = kpads[0].rearrange("p a b c -> p (a b c)")[:, :512]
            for _ in range(9):
                wm = pebcp.tile([P, 512], F32, tag="eb")
                nc.tensor.matmul(wm, ident, wrhs, start=True, stop=True,
                                 skip_group_check=True)

            # ---- pipelined schedule over the two batches ----
            # PE-dense conv chunks are interleaved into the DVE-bound
            # attention phases of the other batch (keeps PE fed and the HAM
            # clock warm).
            load_x(0)
            load_x(1)
            conv1_chunk(0, 0)
            conv1_chunk(0, 1)
            for i in range(6):
                qkv_chunk(0, i)
            logits_all(0, {0: lambda: conv1_chunk(1, 0),
                           1: lambda: conv1_chunk(1, 1)})
            softchain(0)
            qkv_chunk(1, 0)
            qkv_chunk(1, 1)
            qkv_chunk(1, 2)
            acc = ev_start(0, 0)
            for kk in range(NKK):
                ev_kk(0, 0, acc, kk)
                if kk == 2:
                    qkv_chunk(1, 3)
                elif kk == 5:
                    qkv_chunk(1, 4)
                elif kk == 8:
                    qkv_chunk(1, 5)
            ev_end(0, 0, acc)
            acc = ev_start(0, 1)
            logits_all(1, {0: lambda: [ev_kk(0, 1, acc, kk)
                                       for kk in range(4)],
                           1: lambda: [ev_kk(0, 1, acc, kk)
                                       for kk in range(4, NKK)]})
            ev_end(0, 1, acc)
            softchain(1)
            conv3(0, range(0, 5))
            acc = ev_start(1, 0)
            for kk in range(NKK):
                ev_kk(1, 0, acc, kk)
                if kk == 3:
                    conv3(0, [5])
                elif kk == 6:
                    conv3(0, [6])
            ev_end(1, 0, acc)
            acc = ev_start(1, 1)
            for kk in range(NKK):
                ev_kk(1, 1, acc, kk)
                if kk == 4:
                    conv3(0, [7])
            ev_end(1, 1, acc)
            conv3(1, range(OC))

    nc.compile()
    return nc


_PROG = None


def _host_prep(inputs):
    import ml_dtypes
    bf = ml_dtypes.bfloat16
    f = lambda a: np.asarray(a, dtype=np.float32)
    x = f(inputs["x"])
    # fold bn scales into weights (bn(conv(x,W),s,b) = conv(x, s*W) + b)
    w1 = f(inputs["w_conv1"]) * f(inputs["bn1_s"])[:, None]
    wq = f(inputs["wq"]) * f(inputs["bnq_s"])[:, None]
    wk = f(inputs["wk"]) * f(inputs["bnk_s"])[:, None]
    # fold bnatt scale through the (linear) attention-value path into v
    sv = f(inputs["bnatt_s"]) * f(inputs["bnv_s"])
    wv = f(inputs["wv"]) * sv[:, None]
    bv = f(inputs["bnatt_s"]) * f(inputs["bnv_b"])
    w3 = f(inputs["w_conv3"]) * f(inputs["bn3_s"])[:, None]

    posf = (f(inputs["pos_h"]) + f(inputs["pos_w"])).reshape(WIDTH, NKK)

    sel = np.zeros((PT, P, HEADS), np.float32)
    for pt in range(PT):
        for c in range(P):
            sel[pt, c, pt * (P // D) + c // D] = 1.0
    # p2[pt, c, t, 32*j+g] = pos[c_global, 4t+j] if head(c_global)==g
    p2 = np.zeros((PT, P, NT, P), np.float32)
    for pt in range(PT):
        for c in range(P):
            g = pt * (P // D) + c // D
            for kk in range(NKK):
                t, j = kk // 4, kk % 4
                p2[pt, c, t, 32 * j + g] = posf[pt * P + c, kk]
    # sab[r, g] = 1 if r % 32 == g (sum over the 4 packed kk rows)
    sab = np.zeros((P, HEADS), np.float32)
    for r in range(P):
        sab[r, r % HEADS] = 1.0
    # bm[mc, j, r, c] = 1 if r == 32*j + 16*mc + c//8 (head->channel bcast)
    bm = np.zeros((PT, 4, P, P), np.float32)
    for mc in range(PT):
        for j in range(4):
            for c in range(P):
                bm[mc, j, 32 * j + 16 * mc + c // D, c] = 1.0
    com = {
        "w1T": np.ascontiguousarray(w1.T.reshape(KC1, P, WIDTH)).astype(bf),
        "wqT": np.ascontiguousarray(wq.T.reshape(PT, P, WIDTH)).astype(bf),
        "wkT": np.ascontiguousarray(wk.T.reshape(PT, P, WIDTH)).astype(bf),
        "wvT": np.ascontiguousarray(wv.T.reshape(PT, P, WIDTH)).astype(bf),
        "w3T": np.ascontiguousarray(w3.T.reshape(PT, P, OUT)).astype(bf),
        "b1": f(inputs["bn1_b"]).reshape(PT, P, 1),
        "bq": f(inputs["bnq_b"]).reshape(PT, P, 1),
        "bk": f(inputs["bnk_b"]).reshape(PT, P, 1),
        "bv": bv.reshape(PT, P, 1),
        "batt": f(inputs["bnatt_b"]).reshape(PT, P, 1),
        "b3": f(inputs["bn3_b"]).reshape(OC, P, 1),
        "sel": sel.astype(bf),
        "p2": p2.astype(bf),
        "sab": sab.astype(bf),
        "eye32": np.eye(HEADS, dtype=np.float32).astype(bf),
        "ident": np.eye(P, dtype=np.float32).astype(bf),
        "bm": bm.astype(bf),
    }
    xr = x.reshape(B, KC1, P, HW)
    in_maps = []
    for c in range(NC_):
        xs = np.ascontiguousarray(xr[c * BL:(c + 1) * BL])
        in_maps.append(dict(com, x16=xs.astype(bf)))
    return in_maps


def kernel(**inputs):
    global _PROG
    if _PROG is None:
        _PROG = build_program()
    in_maps = _host_prep(inputs)
    res = run_bass_kernel_spmd(_PROG, in_maps, core_ids=list(range(NC_)))
    outs = [np.asarray(res.results[c]["out"], dtype=np.float32)
            .reshape(BL, OUT, H, W) for c in range(NC_)]
    return np.concatenate(outs, axis=0)
